# revision 19
# baseline (speedup 1.0000x reference)
"""Trainium2 Bass kernel for nn_Attention_80917183857290.

Multi-head causal attention (B=2, S=2048, D=1024, H=32, HD=32) with
SSMax-style per-query log-length score scaling, run SPMD on 8 NeuronCores.

Sharding: core c -> batch b = c // 4, head-group g2 = c % 4 (8 heads each).
Per core:
  - projections q,k (transposed layout [head_dim rows, seq]) and v, f32r
    (TF32-like single-pass fp32); x and sll stream in seq-quarters so the
    first attention chunk starts ~15us in; PE-clock warmup dummies burn the
    p-state ramp during the initial DMA.
  - scoresT [128k, 512q] per head per k-tile (f32r, K=32 via tile_position
    quads); a quarter-entry software pipeline (one 1-bank PSUM tile per
    step, 4-deep rotation, scores emitted 2 steps ahead) keeps the
    scores -> exp -> PV chain free of in-order head-of-line stalls.
  - probs = exp(scores * sll * ss / sqrt(hd)): ~4/7 on ACT, ~3/7 on DVE via
    a bf16 Schraudolph bit-trick (tensor_scalar -> int16, bitcast bf16,
    ~2% elementwise error that mostly cancels through softmax).
  - PV with probs as the stationary operand: out [128q, 32hd] per (head,
    kt) accumulated in PSUM across kt -- the narrow free dim makes PV ~4x
    cheaper on the PE than the scoresT-layout PV. Denominators via
    per-head [128q, 1] matmuls against a ones column. The multi-region
    accumulator banks are memset once per qc and accumulated with
    start=False: hardware start=True resets accumulation state at BANK
    granularity and would clobber sibling regions.
  - att = pv * recip(dn) (bf16); att^T via PE transpose (identity matmul);
    out = att^T.T @ wo_shard (f32r); projections/epilogue work is woven
    into the attention steps as paced filler units.
  - host sums the 4 partial outputs per batch.

The causal mask reduces to a single shared [128,128] triangular tile
multiplied only on diagonal-crossing blocks; fully-masked [128 k, 128 q]
blocks are skipped in scores/exp/PV entirely. Non-causal masks fall back
to per-tile bf16 exp(mask) multiplies (correct for any mask).
"""

import math

import numpy as np
import ml_dtypes

B, S, D, H = 2, 2048, 1024, 32
HD = D // H  # 32
P = 128
QC = 512  # q-chunk (PSUM bank free size, fp32)
NQC = S // QC  # 4
NKT = S // P  # 16
NCORES = 8

_GRAPH_CACHE: dict = {}


# exp engine schedule: step_no cycles through EXP_PERIOD phases; phases in
# DVE_PHASES run the Schraudolph exp on DVE, the rest true-exp on ACT.
# GPSIMD (Pool) has no PSUM port, so it cannot read scores; it instead
# absorbs the SBUF-only tril multiplies. Ratio balances ACT/DVE makespans
# (ACT 612ns/step + copies vs DVE 658ns/step + TT/memset work).
EXP_PERIOD = 17
DVE_PHASES = (0, 2, 4, 6, 8, 10, 12, 14, 16)
# bf16 Schraudolph constants: bits = round(x * 128/ln2 + (127*128 - 7.4))
SCH_C1 = 128.0 / math.log(2.0)
SCH_C2 = 127.0 * 128.0 - 7.4


def _build_graph(plans, nt, reps=1):
    """Build the per-core Bass graph.

    plans: tuple over qc (4) of tuple of (kt, mi, zq, tri) entries; mi -1
      means no general-mask multiply; >=0 indexes the packed emask tiles;
      tri means multiply the shared [128,128] triu tile on block zq//128.
      zq is a multiple of 128: leading zq q-columns are fully masked.
    nt: number of packed [128, 512] bf16 exp-mask tiles (>= 1).
    """
    from collections import deque

    import concourse.mybir as mybir
    from concourse import bacc
    from concourse.tile import TileContext

    f32 = mybir.dt.float32
    f32r = mybir.dt.float32r
    bf16 = mybir.dt.bfloat16
    i16 = mybir.dt.int16
    EXP = mybir.ActivationFunctionType.Exp
    MULT = mybir.AluOpType.mult
    ADD = mybir.AluOpType.add

    nc = bacc.Bacc()

    xT = nc.declare_dram_parameter("xT", [D, S], bf16, isOutput=False)
    aq = nc.declare_dram_parameter("aq", [D, 256], bf16, isOutput=False)
    ak = nc.declare_dram_parameter("ak", [D, 256], bf16, isOutput=False)
    av = nc.declare_dram_parameter("av", [D, 256], bf16, isOutput=False)
    wor = nc.declare_dram_parameter("wor", [256, D], bf16, isOutput=False)
    sllb = nc.declare_dram_parameter("sllb", [P, S], f32, isOutput=False)
    tril = nc.declare_dram_parameter("tril", [P, P], bf16, isOutput=False)
    ident = nc.declare_dram_parameter("ident", [P, P], bf16, isOutput=False)
    emask = nc.declare_dram_parameter("emask", [nt, P, QC], bf16, isOutput=False)
    out = nc.declare_dram_parameter("out", [S, D], bf16, isOutput=True)

    # spread "special" entries (tril / masked / zq-skip) evenly among the
    # full-tile entries: their post-exp Pool/DVE multiplies otherwise cluster
    # at the end of each qc and stall the PV chain on the Pool queue
    def _interleave(ent):
        fulls = [e for e in ent if not e[3] and e[1] < 0 and e[2] == 0]
        specials = [e for e in ent if e[3] or e[1] >= 0 or e[2] > 0]
        if not fulls or not specials:
            return tuple(ent)
        n = len(fulls) + len(specials)
        res, fi, si = [], 0, 0
        for i in range(n):
            # Bresenham-style even spread of specials
            if si * n <= i * len(specials) and si < len(specials):
                res.append(specials[si]); si += 1
            elif fi < len(fulls):
                res.append(fulls[fi]); fi += 1
            else:
                res.append(specials[si]); si += 1
        return tuple(res)

    plans = tuple(_interleave(p) for p in plans)

    any_general = any(mi >= 0 for p in plans for (_, mi, _, _) in p)

    # first/last contributing entry index per (qc, qt) for pv/dn accumulation
    first_e = [[None] * 4 for _ in range(NQC)]
    last_e = [[None] * 4 for _ in range(NQC)]
    for qc in range(NQC):
        for ei, (kt, mi, zq, tri) in enumerate(plans[qc]):
            for qt in range(zq // P, 4):
                if first_e[qc][qt] is None:
                    first_e[qc][qt] = ei
                last_e[qc][qt] = ei

    with TileContext(nc) as tc:
        with (
            tc.tile_pool(name="consts", bufs=1) as consts,
            tc.tile_pool(name="sc", bufs=4, space="PSUM") as sc_pool,
            tc.tile_pool(name="pv", bufs=1, space="PSUM") as pv_pool,
            tc.tile_pool(name="ms", bufs=1, space="PSUM") as ms_pool,
            tc.tile_pool(name="ax", bufs=1, space="PSUM") as ax_pool,
            tc.tile_pool(name="probs", bufs=8) as probs_pool,
            tc.tile_pool(name="emt", bufs=4) as emt_pool,
            tc.tile_pool(name="att", bufs=4) as att_pool,
            tc.tile_pool(name="attT", bufs=2) as attT_pool,
            tc.tile_pool(name="oout", bufs=4) as oout_pool,
            tc.tile_pool(name="recip", bufs=4) as recip_pool,
        ):
          for _rep in range(reps):
            # ---- constant loads, ordered by first use; xT and sll stream in
            # seq-quarters so the first attention chunk starts ~15us in ----
            # PE p-state warmup: dummy wide matmuls (no DMA deps) burn the
            # clock-ramp window while the first input DMAs are in flight
            ones_sb = consts.tile([P, 1], bf16, tag="ones", name="ones")
            nc.gpsimd.memset(ones_sb[:], 1.0)
            dumm = consts.tile([P, 512], bf16, tag="dumm", name="dumm")
            nc.gpsimd.memset(dumm[:], 0.0)
            # force the Exp table load onto ACT now, so it overlaps the input
            # DMA window instead of stalling the first k-proj copy
            actw = consts.tile([P, 1], f32, tag="actw", name="actw")
            nc.scalar.activation(actw[:], ones_sb[:], EXP)
            wps = ax_pool.tile([P, 512], f32, tag="ax", name="ax")
            for _ in range(9):
                nc.tensor.matmul(
                    wps[0:1, :],
                    lhsT=ones_sb[:],
                    rhs=dumm[:],
                    start=True,
                    stop=True,
                    skip_group_check=True,
                )

            aq_sb = consts.tile([P, 8, 256], bf16, tag="aq", name="aq")
            aq_r = aq.rearrange("(ko ki) f -> ki ko f", ki=P)
            nc.sync.dma_start(out=aq_sb[:, :, 0:128], in_=aq_r[:, :, 0:128])

            sll_sb = consts.tile([P, 4, QC], f32, tag="sll", name="sll")
            xall = consts.tile([P, 8, 4, QC], bf16, tag="xall", name="xall")
            xT_r = xT.rearrange("(ko ki) f -> ki ko f", ki=P)

            def load_quarter(qtr):
                # batched DMA per 4 contraction chunks: proj h0 (k 0-3) can
                # start before the second half lands
                nc.sync.dma_start(
                    out=xall[:, 0:4, qtr], in_=xT_r[:, 0:4, QC * qtr : QC * (qtr + 1)]
                )
                nc.sync.dma_start(
                    out=xall[:, 4:8, qtr], in_=xT_r[:, 4:8, QC * qtr : QC * (qtr + 1)]
                )
                nc.sync.dma_start(
                    out=sll_sb[:, qtr], in_=sllb[:, QC * qtr : QC * (qtr + 1)]
                )

            load_quarter(0)
            ak_sb = consts.tile([P, 8, 256], bf16, tag="ak", name="ak")
            ak_r = ak.rearrange("(ko ki) f -> ki ko f", ki=P)
            nc.sync.dma_start(out=ak_sb[:, :, 0:128], in_=ak_r[:, :, 0:128])
            tril_sb = consts.tile([P, P], bf16, tag="tril", name="tril")
            nc.sync.dma_start(out=tril_sb[:], in_=tril[:])
            em_res = None
            if any_general and nt <= 16:
                em_res = consts.tile([P, nt, QC], bf16, tag="emres", name="emres")
                nc.sync.dma_start(out=em_res[:], in_=emask.rearrange("t p f -> p t f"))
            nc.sync.dma_start(out=aq_sb[:, :, 128:256], in_=aq_r[:, :, 128:256])
            nc.sync.dma_start(out=ak_sb[:, :, 128:256], in_=ak_r[:, :, 128:256])
            av_sb = consts.tile([P, 8, 256], bf16, tag="av", name="av")
            nc.sync.dma_start(out=av_sb[:], in_=av.rearrange("(ko ki) f -> ki ko f", ki=P))
            load_quarter(1)
            id_sb = consts.tile([P, P], bf16, tag="ident", name="ident")
            nc.sync.dma_start(out=id_sb[:], in_=ident[:])
            wor_sb = consts.tile([P, 2, D], bf16, tag="wor", name="wor")
            nc.sync.dma_start(out=wor_sb[:], in_=wor.rearrange("(ko ki) f -> ki ko f", ki=P))
            load_quarter(2)
            load_quarter(3)

            # persistent PSUM accumulators (slice-granular reuse across qc)
            pv01 = pv_pool.tile([P, 2, 8, 32], f32, tag="pv01", name="pv01")
            pv23 = pv_pool.tile([P, 2, 8, 32], f32, tag="pv23", name="pv23")
            ms = ms_pool.tile([P, 512], f32, tag="ms", name="ms")
            ms_b = ms[:].bitcast(bf16)  # [P, 1024]: psT regions at 256/512

            qTh = [[consts.tile([P, 1024], bf16, tag=f"qT{g}{h}", name=f"qT{g}{h}") for h in range(2)] for g in range(2)]
            kTh = [[consts.tile([P, 1024], bf16, tag=f"kT{g}{h}", name=f"kT{g}{h}") for h in range(2)] for g in range(2)]
            vq = [consts.tile([P, 4, 8, 32], bf16, tag=f"vq{q}", name=f"vq{q}") for q in range(4)]

            def proj_qk_halves(w, g, half, c2):
                # one 512-wide chunk of the q or k projection for (g, half),
                # split into two 4-deep accumulation halves so filler pops
                # stay under the ACT step time
                lhs_sb = aq_sb if w == "q" else ak_sb
                qtr = 2 * half + c2
                state = {}

                def mm(ps, k):
                    nc.tensor.matmul(
                        ps[:],
                        lhsT=lhs_sb[:, k, 128 * g : 128 * g + 128],
                        rhs=xall[:, k, qtr],
                        start=(k == 0),
                        stop=(k == 7),
                        skip_group_check=True,
                    )

                def h0():
                    state["ps"] = ax_pool.tile([P, 512], f32, tag="ax", name="ax")
                    for k in range(4):
                        mm(state["ps"], k)

                def h1():
                    ps = state["ps"]
                    for k in range(4, 8):
                        mm(ps, k)
                    dst = (qTh if w == "q" else kTh)[g][half][:, 512 * c2 : 512 * c2 + 512]
                    if w == "q":
                        nc.vector.tensor_tensor(dst, ps[:], sll_sb[:, qtr], MULT)
                    else:
                        nc.scalar.copy(dst, ps[:])

                return [h0, h1]

            def proj_v_halves(st):
                qq, sti = st // 4, st % 4
                state = {}

                def mm(psv, k):
                    nc.tensor.matmul(
                        psv[:, :256],
                        lhsT=xall[:, k, qq, 128 * sti : 128 * sti + 128],
                        rhs=av_sb[:, k, :],
                        start=(k == 0),
                        stop=(k == 7),
                        skip_group_check=True,
                    )

                def h0():
                    state["ps"] = ax_pool.tile([P, 512], f32, tag="ax", name="ax")
                    for k in range(4):
                        mm(state["ps"], k)

                def h1():
                    psv = state["ps"]
                    for k in range(4, 8):
                        mm(psv, k)
                    nc.scalar.copy(
                        vq[qq][:, sti, :, :],
                        psv[:, :256].rearrange("p (h c) -> p h c", h=8),
                    )

                return [h0, h1]

            def make_groups(r):
                # each group is a mutable list of halves sharing one ax tile;
                # halves of a group must emit with no other ax alloc between
                if r[0] == "qk":
                    _, half, c2 = r
                    return [
                        proj_qk_halves(w, g, half, c2)
                        for g in range(2)
                        for w in ("q", "k")
                    ]
                _, st = r
                return [proj_v_halves(st)]

            done: set = set()
            fillers: deque = deque()  # of group lists
            pending: deque = deque()  # forced-next second halves
            epi: deque = deque()
            group_of: dict = {}
            rr = [0]

            def emit_group_now(gr):
                while gr:
                    gr.pop(0)()

            def drain(r):
                # emit any not-yet-emitted units of resource r immediately
                while pending:
                    pending.popleft()()
                if r not in done:
                    done.add(r)
                    for gr in make_groups(r):
                        emit_group_now(gr)
                    return
                for gr in group_of.pop(r, []):
                    try:
                        fillers.remove(gr)
                    except ValueError:
                        pass  # already popped (possibly partially emitted)
                    emit_group_now(gr)

            def queue(r):
                if r in done:
                    return
                done.add(r)
                grs = make_groups(r)
                group_of[r] = grs
                fillers.extend(grs)

            def pop_work():
                if pending:
                    pending.popleft()()
                    return
                rr[0] += 1
                if rr[0] % 3 != 2 and epi:
                    epi.popleft()[1]()
                    return
                if fillers:
                    gr = fillers.popleft()
                    gr.pop(0)()
                    if gr:
                        pending.append(gr.pop(0))
                    return
                if epi:
                    epi.popleft()[1]()

            step_no = [0]
            drain_mode = [False]

            def attention_qc(qc):
                entries = plans[qc]
                qh, qcol = qc // 2, 512 * (qc % 2)

                ats = [None] * 4
                aTs = [None] * 4

                def mk_divide(qt):
                    def _div():
                        rc = recip_pool.tile([P, 8], f32, tag="rc", name="rc")
                        nc.vector.reciprocal_approx_fast(
                            out=rc[:], in_=ms[:, 8 * qt : 8 * qt + 8]
                        )
                        at = att_pool.tile([P, 8, 32], bf16, tag="at", name="at")
                        nc.vector.tensor_tensor(
                            at[:],
                            (pv01 if qt < 2 else pv23)[:, qt % 2],
                            rc[:, :, None].to_broadcast((P, 8, 32)),
                            MULT,
                        )
                        ats[qt] = at

                    return _div

                def mk_transpose(qt):
                    def _tr():
                        if drain_mode[0]:
                            axt = sc_pool.tile([P, 512], f32, tag="sc", name="sc")
                        else:
                            axt = ax_pool.tile([P, 512], f32, tag="ax", name="ax")
                        psT = axt[:].bitcast(bf16)[:, 0:256].rearrange("p (c q) -> p c q", c=2)
                        at2 = ats[qt][:].rearrange("p h c -> p (h c)")
                        for c in range(2):
                            nc.tensor.matmul(
                                psT[:, c],
                                lhsT=at2[:, 128 * c : 128 * c + 128],
                                rhs=id_sb[:],
                                is_transpose=True,
                            )
                        aT = attT_pool.tile([P, 2, 128], bf16, tag="aT", name="aT")
                        nc.vector.tensor_copy(aT[:], psT)
                        aTs[qt] = aT

                    return _tr

                def mk_wo(qt, n):
                    def _wo():
                        aT = aTs[qt]
                        st = 4 * qc + qt
                        if drain_mode[0]:
                            # final drain: scores are done, rotate wo psum
                            # through the free sc banks so consecutive wo
                            # matmuls don't serialize on the single ax bank
                            wops = sc_pool.tile([P, 512], f32, tag="sc", name="sc")
                        else:
                            wops = ax_pool.tile([P, 512], f32, tag="ax", name="ax")
                        for kk in range(2):
                            nc.tensor.matmul(
                                wops[:],
                                lhsT=aT[:, kk],
                                rhs=wor_sb[:, kk, 512 * n : 512 * n + 512],
                                start=(kk == 0),
                                stop=(kk == 1),
                            )
                        ob = oout_pool.tile([P, 512], bf16, tag="ob", name="ob")
                        if drain_mode[0] and n == 1:
                            nc.vector.tensor_copy(ob[:], wops[:])
                        else:
                            nc.scalar.copy(ob[:], wops[:])
                        nc.sync.dma_start(
                            out=out[128 * st : 128 * st + 128, 512 * n : 512 * n + 512],
                            in_=ob[:],
                        )

                    return _wo

                if not entries:
                    for qt in range(4):
                        at = att_pool.tile([P, 8, 32], bf16, tag="at", name="at")
                        nc.vector.memset(at[:], 0.0)
                        # transpose of zeros is zeros; emit wo directly on it
                        axt = ax_pool.tile([P, 512], f32, tag="ax", name="ax")
                        psT = axt[:].bitcast(bf16)[:, 0:256].rearrange("p (c q) -> p c q", c=2)
                        at2 = at[:].rearrange("p h c -> p (h c)")
                        for c in range(2):
                            nc.tensor.matmul(
                                psT[:, c],
                                lhsT=at2[:, 128 * c : 128 * c + 128],
                                rhs=id_sb[:],
                                is_transpose=True,
                            )
                        aT = attT_pool.tile([P, 2, 128], bf16, tag="aT", name="aT")
                        nc.vector.tensor_copy(aT[:], psT)
                        aTs[qt] = aT
                        epi.append(("wo", mk_wo(qt, 0)))
                        epi.append(("wo", mk_wo(qt, 1)))
                    return

                def pv_dn(g, jq, ei, kt, zq, pr):
                    # PV + denominator matmuls for an already-exp'd quarter.
                    # start=False always: on hardware, start=True resets the
                    # accumulation epoch at BANK granularity, clobbering the
                    # other interleaved (qt, h) regions -- so the banks are
                    # memset once per qc and every matmul accumulates.
                    for jj in range(1):
                        h = 4 * g + jq
                        for qt in range(zq // P, 4):
                            lhsT = pr[:, 128 * qt : 128 * qt + 128]
                            pvd = (pv01 if qt < 2 else pv23)[:, qt % 2, h, :]
                            nc.tensor.matmul(
                                pvd,
                                lhsT=lhsT,
                                rhs=vq[kt // 4][:, kt % 4, h, :],
                                start=False,
                                stop=(ei == last_e[qc][qt]),
                                skip_group_check=True,
                            )
                            nc.tensor.matmul(
                                ms[:, 8 * qt + h : 8 * qt + h + 1],
                                lhsT=lhsT,
                                rhs=ones_sb[:],
                                start=False,
                                stop=(ei == last_e[qc][qt]),
                                skip_group_check=True,
                            )
                    if g == 1 and jq == 3:
                        for qt in range(4):
                            if ei == last_e[qc][qt]:
                                epi.append(("div", mk_divide(qt)))
                                epi.append(("tr", mk_transpose(qt)))
                                epi.append(("wo", mk_wo(qt, 0)))
                                epi.append(("wo", mk_wo(qt, 1)))

                # software-pipelined emission: scores+exp of step s+2 are
                # emitted BEFORE pv/dn of step s, so the in-order PE stream
                # never head-of-line blocks on the exp of the current entry.
                # quarter-entry software pipeline: each step covers ONE head
                # of one (g, entry) in ONE 1-bank [P,512] sc tile. sc bufs=4
                # gives a 4-deep rotation, so a late DVE-Schraudolph exp (its
                # queue holds other work) never stalls the ACT exp chain; the
                # exp engine is chosen per step (~1/3 DVE) to balance
                # makespans. Scores are emitted two steps ahead of their exp
                # so they precede the pv/filler bursts in the in-order PE
                # stream.
                hsteps = [
                    (g, jq, ei, kt, mi, zq, tri)
                    for ei, (kt, mi, zq, tri) in enumerate(entries)
                    for g in range(2)
                    for jq in range(4)
                ]

                def sc_emit(h):
                    g, jq, ei, kt, mi, zq, tri = h
                    kh, kcol = kt // 8, 128 * (kt % 8)
                    szq = zq  # bf16 matmul: no small-free-dim penalty
                    psc = sc_pool.tile([P, 512], f32, tag="sc", name="sc")
                    nc.tensor.matmul(
                        psc[:, szq:512],
                        lhsT=kTh[g][kh][32 * jq : 32 * jq + 32, kcol : kcol + 128],
                        rhs=qTh[g][qh][32 * jq : 32 * jq + 32, qcol + szq : qcol + 512],
                        start=True,
                        stop=True,
                        tile_position=(32 * jq, 0),
                    )
                    return psc

                def exp_emit(h, psc):
                    g, jq, ei, kt, mi, zq, tri = h
                    # scores pad the matmul to >=256 wide, but only [zq:] is
                    # ever read downstream -- exp just that
                    pr = probs_pool.tile([P, 512], bf16, tag="pr", name="pr")
                    ph = step_no[0] % EXP_PERIOD
                    step_no[0] += 1
                    if ph in DVE_PHASES:
                        nc.vector.tensor_scalar(
                            pr[:].bitcast(i16)[:, zq:], psc[:, zq:],
                            SCH_C1, SCH_C2, MULT, ADD,
                        )
                    else:
                        nc.scalar.activation(pr[:, zq:], psc[:, zq:], EXP)
                    if tri:
                        # SBUF-only bf16 multiply: offload to the otherwise
                        # idle GPSIMD (Pool) engine
                        blk = pr[:, zq : zq + P]
                        nc.gpsimd.tensor_tensor(blk, blk, tril_sb[:], MULT)
                    elif mi >= 0:
                        if em_res is not None:
                            emt = em_res[:, mi, :]
                        else:
                            emtt = emt_pool.tile([P, QC], bf16, tag="emt", name="emt")
                            nc.sync.dma_start(out=emtt[:], in_=emask[mi])
                            emt = emtt[:]
                        nc.vector.tensor_tensor(
                            pr[:, zq:], pr[:, zq:], emt[:, zq:], MULT
                        )
                    return pr

                if any(k == "div" for k, _ in epi):
                    rest = deque()
                    while epi:
                        k, fn = epi.popleft()
                        if k == "div":
                            fn()
                        else:
                            rest.append((k, fn))
                    epi.extend(rest)
                nc.vector.memset(pv01[:], 0.0)
                nc.vector.memset(pv23[:], 0.0)
                nc.vector.memset(ms[:, :32], 0.0)

                ns = len(hsteps)
                pscs: dict = {}
                prs: dict = {}
                for s in range(ns):
                    if s == 0:
                        pscs[0] = sc_emit(hsteps[0])
                        if ns > 1:
                            pscs[1] = sc_emit(hsteps[1])
                    if s + 2 < ns:
                        pscs[s + 2] = sc_emit(hsteps[s + 2])
                    prs[s] = exp_emit(hsteps[s], pscs.pop(s))
                    if s >= 3:
                        h = hsteps[s - 3]
                        drain(("v", h[3]))
                        pv_dn(h[0], h[1], h[2], h[3], h[5], prs.pop(s - 3))
                    if qc == NQC - 1 or s % 2 == 0 or len(epi) + len(fillers) > 8:
                        pop_work()
                for s in (ns - 3, ns - 2, ns - 1):
                    if s < 0 or s not in prs:
                        continue
                    h = hsteps[s]
                    drain(("v", h[3]))
                    pv_dn(h[0], h[1], h[2], h[3], h[5], prs.pop(s))
                    pop_work()

            def needs(qc):
                res = [("qk", qc // 2, qc % 2)]
                for kt, _, _, _ in plans[qc]:
                    r = ("qk", kt // 8, (kt % 8) // 4)
                    if r not in res:
                        res.append(r)
                return res

            for qc in range(NQC):
                for r in needs(qc):
                    drain(r)
                for kt, _, _, _ in plans[qc]:
                    queue(("v", kt))
                if qc + 1 < NQC:
                    for r in needs(qc + 1):
                        queue(r)
                    for kt, _, _, _ in plans[qc + 1]:
                        queue(("v", kt))
                attention_qc(qc)
            # final drain: divides/transposes first so the wo chains overlap;
            # wo psum rotates through the now-idle sc banks
            drain_mode[0] = True
            _order = {"div": 0, "tr": 1}
            _rest = sorted(epi, key=lambda kv: _order.get(kv[0], 2))
            epi.clear()
            epi.extend(_rest)
            while epi or fillers:
                pop_work()

    if not nc.is_finalized():
        nc.finalize()
    return nc


def _round_f32r(a):
    """Round fp32 array to the PE's f32r format (mantissa truncated to 11
    bits, round-to-nearest-even at bit 12) so f32r-declared DMA inputs match
    what an on-device cast would produce."""
    u = np.ascontiguousarray(a, dtype=np.float32).view(np.uint32)
    u2 = (u + np.uint32(0x7FF) + ((u >> np.uint32(12)) & np.uint32(1))) & np.uint32(0xFFFFF000)
    return u2.view(np.float32)


def _plan_from_mask(mask):
    """Classify [128, 512] tiles of exp(mask)^T; returns (plans, packed_tiles).

    Entries are (kt, mi, zq, tri): zq (multiple of 128) leading fully-masked
    q-columns; tri=True means the tile is [zeros | tril(128) | ones] so only
    the shared tril block needs multiplying; mi >= 0 indexes a packed general
    bf16 exp(mask) tile.
    """
    em = np.exp(mask.astype(np.float32))  # [q, k]
    emT = np.ascontiguousarray(em.T)  # [k, q]
    # partial diagonal block in [k, q] layout: valid iff q_local >= k_local
    tril_blk = np.triu(np.ones((P, P), dtype=np.float32))
    plans = []
    tiles = []
    tile_keys = {}
    for qc in range(NQC):
        ent = []
        covered = [False] * 4
        for kt in range(NKT):
            t = emT[P * kt : P * (kt + 1), QC * qc : QC * (qc + 1)]
            if not t.any():
                continue  # fully masked out: skip tile entirely
            if (t == 1.0).all():
                ent.append((kt, -1, 0, False))
                continue
            nz = np.flatnonzero(t.any(axis=0))
            zq = (int(nz[0]) // P) * P
            # tril-structured tile: [zeros(zq) | tril | ones]
            tri = (
                zq + P <= QC
                and (t[:, :zq] == 0.0).all()
                and (t[:, zq : zq + P] == tril_blk).all()
                and (t[:, zq + P :] == 1.0).all()
            )
            if tri:
                ent.append((kt, -1, zq, True))
                continue
            key = t.tobytes()
            mi = tile_keys.get(key)
            if mi is None:
                mi = len(tiles)
                tile_keys[key] = mi
                tiles.append(t.astype(ml_dtypes.bfloat16))
            ent.append((kt, mi, zq, False))
        for kt, mi, zq, tri in ent:
            for qt in range(zq // P, 4):
                covered[qt] = True
        if ent and not all(covered):
            # some qt block would never be written: disable skipping (the
            # emask multiply zeroes masked probs so pv/dn stay correct)
            ent2 = []
            for kt, mi, zq, tri in ent:
                if zq == 0:
                    ent2.append((kt, mi, zq, tri))
                    continue
                t = emT[P * kt : P * (kt + 1), QC * qc : QC * (qc + 1)]
                key = t.tobytes()
                mi = tile_keys.get(key)
                if mi is None:
                    mi = len(tiles)
                    tile_keys[key] = mi
                    tiles.append(t.astype(ml_dtypes.bfloat16))
                ent2.append((kt, mi, 0, False))
            ent = ent2
        plans.append(tuple(ent))
    if tiles:
        packed = np.ascontiguousarray(np.stack(tiles))
    else:
        packed = np.zeros((1, P, QC), dtype=ml_dtypes.bfloat16)
    return tuple(plans), packed


def kernel(x, mask, section_log_len, wq, wk, wv, wo, seq_scale):
    from concourse.bass_utils import run_bass_kernel_spmd

    x = np.asarray(x, dtype=np.float32)
    assert x.shape == (B, S, D), x.shape
    mask2 = np.asarray(mask, dtype=np.float32).reshape(S, S)
    sll = np.asarray(section_log_len, dtype=np.float32).reshape(S)
    ss = np.asarray(seq_scale, dtype=np.float32).reshape(H)
    wq = np.asarray(wq, dtype=np.float32)
    wk = np.asarray(wk, dtype=np.float32)
    wv = np.asarray(wv, dtype=np.float32)
    wo = np.asarray(wo, dtype=np.float32)

    plans, tiles = _plan_from_mask(mask2)
    key = (plans, tiles.shape[0])
    nc = _GRAPH_CACHE.get(key)
    if nc is None:
        nc = _build_graph(plans, tiles.shape[0])
        _GRAPH_CACHE[key] = nc

    bf = ml_dtypes.bfloat16
    sllB = np.ascontiguousarray(
        np.broadcast_to(sll[None, :], (P, S)), dtype=np.float32
    )
    xT = [np.ascontiguousarray(x[b].T).astype(bf) for b in range(B)]
    trilB = np.triu(np.ones((P, P), dtype=np.float32)).astype(bf)
    identB = np.eye(P, dtype=np.float32).astype(bf)

    in_maps = []
    for c in range(NCORES):
        b, g2 = divmod(c, 4)
        rows = slice(256 * g2, 256 * (g2 + 1))
        ssr = np.repeat(ss[8 * g2 : 8 * g2 + 8], HD) / math.sqrt(HD)
        in_maps.append(
            {
                "xT": xT[b],
                "aq": np.ascontiguousarray((wq[rows, :] * ssr[:, None]).T).astype(bf),
                "ak": np.ascontiguousarray(wk[rows, :].T).astype(bf),
                "av": np.ascontiguousarray(wv[rows, :].T).astype(bf),
                "wor": np.ascontiguousarray(wo[:, rows].T).astype(bf),
                "sllb": sllB,
                "tril": trilB,
                "ident": identB,
                "emask": tiles,
            }
        )

    res = run_bass_kernel_spmd(nc, in_maps, core_ids=list(range(NCORES))).results
    out = np.zeros((B, S, D), dtype=np.float32)
    for c in range(NCORES):
        out[c // 4] += np.asarray(res[c]["out"], dtype=np.float32)
    return out



# revision 27
# speedup vs baseline: 1.0443x; 1.0443x over previous
"""Trainium2 Bass kernel for nn_Attention_80917183857290.

Multi-head causal attention (B=2, S=2048, D=1024, H=32, HD=32) with
SSMax-style per-query log-length score scaling, run SPMD on 8 NeuronCores.

Sharding: core c -> batch b = c // 4, head-group g2 = c % 4 (8 heads each).
Per core:
  - projections q,k (transposed layout [head_dim rows, seq]) and v, f32r
    (TF32-like single-pass fp32); x and sll stream in seq-quarters so the
    first attention chunk starts ~15us in; PE-clock warmup dummies burn the
    p-state ramp during the initial DMA.
  - scoresT [128k, 512q] per head per k-tile (f32r, K=32 via tile_position
    quads); a quarter-entry software pipeline (one 1-bank PSUM tile per
    step, 4-deep rotation, scores emitted 2 steps ahead) keeps the
    scores -> exp -> PV chain free of in-order head-of-line stalls.
  - probs = exp(scores * sll * ss / sqrt(hd)): ~4/7 on ACT, ~3/7 on DVE via
    a bf16 Schraudolph bit-trick (tensor_scalar -> int16, bitcast bf16,
    ~2% elementwise error that mostly cancels through softmax).
  - PV with probs as the stationary operand: out [128q, 32hd] per (head,
    kt) accumulated in PSUM across kt -- the narrow free dim makes PV ~4x
    cheaper on the PE than the scoresT-layout PV. Denominators via
    per-head [128q, 1] matmuls against a ones column. The multi-region
    accumulator banks are memset once per qc and accumulated with
    start=False: hardware start=True resets accumulation state at BANK
    granularity and would clobber sibling regions.
  - att = pv * recip(dn) (bf16); att^T via PE transpose (identity matmul);
    out = att^T.T @ wo_shard (f32r); projections/epilogue work is woven
    into the attention steps as paced filler units.
  - host sums the 4 partial outputs per batch.

The causal mask reduces to a single shared [128,128] triangular tile
multiplied only on diagonal-crossing blocks; fully-masked [128 k, 128 q]
blocks are skipped in scores/exp/PV entirely. Non-causal masks fall back
to per-tile bf16 exp(mask) multiplies (correct for any mask).
"""

import math

import numpy as np
import ml_dtypes

B, S, D, H = 2, 2048, 1024, 32
HD = D // H  # 32
P = 128
QC = 512  # q-chunk (PSUM bank free size, fp32)
NQC = S // QC  # 4
NKT = S // P  # 16
NCORES = 8

_GRAPH_CACHE: dict = {}


# exp engine schedule: step_no cycles through EXP_PERIOD phases; phases in
# DVE_PHASES run the Schraudolph exp on DVE, the rest true-exp on ACT.
# GPSIMD (Pool) has no PSUM port, so it cannot read scores; it instead
# absorbs the SBUF-only tril multiplies. Ratio balances ACT/DVE makespans
# (ACT 612ns/step + copies vs DVE 658ns/step + TT/memset work).
EXP_PERIOD = 17
DVE_PHASES = (0, 2, 4, 6, 8, 10, 12, 14, 16)

# scheduling knobs (A/B-tunable from bench.py)
KNOBS = {
    "tril_engine": "pool",   # pool | dve | follow (dve after dve-exp, pool after act-exp)
    "interleave": False,       # spread diag entries among full entries
    "pv_lag": 2,              # steps between exp and pv consumption
    "aggr_last": True,        # pop_work every step in the last qc
    "dummies": 9,             # warmup matmul count
    "x_split": 2,             # DMAs per x quarter
    "drain_sc": True,         # wo/tr psum from sc banks during final drain
    "ob_drain_alt": True,     # alternate ob copy engine in final drain
    "exp_period": 17,
    "dve_phases": (0, 2, 4, 6, 8, 10, 12, 14, 16),
    "dve_phases_qc": None,     # optional per-qc override: tuple of 4 phase-tuples
    "qc_order": (0, 1, 2, 3),  # processing order of q-chunks
    "ob_alt": False,           # alternate ob copy engine globally
    "vq_alt": False,           # alternate vq copy engine
    "div_merge": False,         # one divide per qt-pair instead of per qt
}
# bf16 Schraudolph constants: bits = round(x * 128/ln2 + (127*128 - 7.4))
SCH_C1 = 128.0 / math.log(2.0)
SCH_C2 = 127.0 * 128.0 - 7.4


def _build_graph(plans, nt, reps=1):
    """Build the per-core Bass graph.

    plans: tuple over qc (4) of tuple of (kt, mi, zq, tri) entries; mi -1
      means no general-mask multiply; >=0 indexes the packed emask tiles;
      tri means multiply the shared [128,128] triu tile on block zq//128.
      zq is a multiple of 128: leading zq q-columns are fully masked.
    nt: number of packed [128, 512] bf16 exp-mask tiles (>= 1).
    """
    from collections import deque

    import concourse.mybir as mybir
    from concourse import bacc
    from concourse.tile import TileContext

    f32 = mybir.dt.float32
    f32r = mybir.dt.float32r
    bf16 = mybir.dt.bfloat16
    i16 = mybir.dt.int16
    EXP = mybir.ActivationFunctionType.Exp
    MULT = mybir.AluOpType.mult
    ADD = mybir.AluOpType.add

    nc = bacc.Bacc()

    xT = nc.declare_dram_parameter("xT", [D, S], bf16, isOutput=False)
    aq = nc.declare_dram_parameter("aq", [D, 256], bf16, isOutput=False)
    ak = nc.declare_dram_parameter("ak", [D, 256], bf16, isOutput=False)
    av = nc.declare_dram_parameter("av", [D, 256], bf16, isOutput=False)
    wor = nc.declare_dram_parameter("wor", [256, D], bf16, isOutput=False)
    sllb = nc.declare_dram_parameter("sllb", [P, S], f32, isOutput=False)
    tril = nc.declare_dram_parameter("tril", [P, P], bf16, isOutput=False)
    ident = nc.declare_dram_parameter("ident", [P, P], bf16, isOutput=False)
    emask = nc.declare_dram_parameter("emask", [nt, P, QC], bf16, isOutput=False)
    out = nc.declare_dram_parameter("out", [S, D], bf16, isOutput=True)

    # spread "special" entries (tril / masked / zq-skip) evenly among the
    # full-tile entries: their post-exp Pool/DVE multiplies otherwise cluster
    # at the end of each qc and stall the PV chain on the Pool queue
    def _interleave(ent):
        fulls = [e for e in ent if not e[3] and e[1] < 0 and e[2] == 0]
        specials = [e for e in ent if e[3] or e[1] >= 0 or e[2] > 0]
        if not fulls or not specials:
            return tuple(ent)
        n = len(fulls) + len(specials)
        res, fi, si = [], 0, 0
        for i in range(n):
            # Bresenham-style even spread of specials
            if si * n <= i * len(specials) and si < len(specials):
                res.append(specials[si]); si += 1
            elif fi < len(fulls):
                res.append(fulls[fi]); fi += 1
            else:
                res.append(specials[si]); si += 1
        return tuple(res)

    if KNOBS["interleave"]:
        plans = tuple(_interleave(p) for p in plans)

    any_general = any(mi >= 0 for p in plans for (_, mi, _, _) in p)

    # first/last contributing entry index per (qc, qt) for pv/dn accumulation
    first_e = [[None] * 4 for _ in range(NQC)]
    last_e = [[None] * 4 for _ in range(NQC)]
    for qc in range(NQC):
        for ei, (kt, mi, zq, tri) in enumerate(plans[qc]):
            for qt in range(zq // P, 4):
                if first_e[qc][qt] is None:
                    first_e[qc][qt] = ei
                last_e[qc][qt] = ei

    with TileContext(nc) as tc:
        with (
            tc.tile_pool(name="consts", bufs=1) as consts,
            tc.tile_pool(name="sc", bufs=4, space="PSUM") as sc_pool,
            tc.tile_pool(name="pv", bufs=1, space="PSUM") as pv_pool,
            tc.tile_pool(name="ms", bufs=1, space="PSUM") as ms_pool,
            tc.tile_pool(name="ax", bufs=1, space="PSUM") as ax_pool,
            tc.tile_pool(name="probs", bufs=8) as probs_pool,
            tc.tile_pool(name="emt", bufs=4) as emt_pool,
            tc.tile_pool(name="att", bufs=4) as att_pool,
            tc.tile_pool(name="attT", bufs=2) as attT_pool,
            tc.tile_pool(name="oout", bufs=4) as oout_pool,
            tc.tile_pool(name="recip", bufs=4) as recip_pool,
        ):
          for _rep in range(reps):
            # ---- constant loads, ordered by first use; xT and sll stream in
            # seq-quarters so the first attention chunk starts ~15us in ----
            # PE p-state warmup: dummy wide matmuls (no DMA deps) burn the
            # clock-ramp window while the first input DMAs are in flight
            ones_sb = consts.tile([P, 1], bf16, tag="ones", name="ones")
            nc.vector.memset(ones_sb[:], 1.0)
            dumm = consts.tile([P, 512], bf16, tag="dumm", name="dumm")
            nc.vector.memset(dumm[:], 0.0)
            # force the Exp table load onto ACT now, so it overlaps the input
            # DMA window instead of stalling the first k-proj copy
            actw = consts.tile([P, 1], f32, tag="actw", name="actw")
            nc.scalar.activation(actw[:], ones_sb[:], EXP)
            wps = ax_pool.tile([P, 512], f32, tag="ax", name="ax")
            for _ in range(KNOBS["dummies"]):
                nc.tensor.matmul(
                    wps[0:1, :],
                    lhsT=ones_sb[:],
                    rhs=dumm[:],
                    start=True,
                    stop=True,
                    skip_group_check=True,
                )

            aq_sb = consts.tile([P, 8, 256], bf16, tag="aq", name="aq")
            aq_r = aq.rearrange("(ko ki) f -> ki ko f", ki=P)
            nc.sync.dma_start(out=aq_sb[:, :, 0:128], in_=aq_r[:, :, 0:128])

            sll_sb = consts.tile([P, 4, QC], f32, tag="sll", name="sll")
            xall = consts.tile([P, 8, 4, QC], bf16, tag="xall", name="xall")
            xT_r = xT.rearrange("(ko ki) f -> ki ko f", ki=P)

            def load_quarter(qtr):
                # batched DMA per 8/xs contraction chunks: proj h0 (k 0-3)
                # can start before later chunks land
                xs = KNOBS["x_split"]
                for j in range(xs):
                    k0, k1 = 8 * j // xs, 8 * (j + 1) // xs
                    nc.sync.dma_start(
                        out=xall[:, k0:k1, qtr],
                        in_=xT_r[:, k0:k1, QC * qtr : QC * (qtr + 1)],
                    )
                nc.sync.dma_start(
                    out=sll_sb[:, qtr], in_=sllb[:, QC * qtr : QC * (qtr + 1)]
                )

            load_quarter(0)
            ak_sb = consts.tile([P, 8, 256], bf16, tag="ak", name="ak")
            ak_r = ak.rearrange("(ko ki) f -> ki ko f", ki=P)
            nc.sync.dma_start(out=ak_sb[:, :, 0:128], in_=ak_r[:, :, 0:128])
            nc.sync.dma_start(out=aq_sb[:, :, 128:256], in_=aq_r[:, :, 128:256])
            nc.sync.dma_start(out=ak_sb[:, :, 128:256], in_=ak_r[:, :, 128:256])
            tril_sb = consts.tile([P, P], bf16, tag="tril", name="tril")
            nc.sync.dma_start(out=tril_sb[:], in_=tril[:])
            em_res = None
            if any_general and nt <= 16:
                em_res = consts.tile([P, nt, QC], bf16, tag="emres", name="emres")
                nc.sync.dma_start(out=em_res[:], in_=emask.rearrange("t p f -> p t f"))
            av_sb = consts.tile([P, 8, 256], bf16, tag="av", name="av")
            nc.sync.dma_start(out=av_sb[:], in_=av.rearrange("(ko ki) f -> ki ko f", ki=P))
            load_quarter(1)
            id_sb = consts.tile([P, P], bf16, tag="ident", name="ident")
            nc.sync.dma_start(out=id_sb[:], in_=ident[:])
            wor_sb = consts.tile([P, 2, D], bf16, tag="wor", name="wor")
            nc.sync.dma_start(out=wor_sb[:], in_=wor.rearrange("(ko ki) f -> ki ko f", ki=P))
            load_quarter(2)
            load_quarter(3)

            # persistent PSUM accumulators (slice-granular reuse across qc)
            pv01 = pv_pool.tile([P, 2, 8, 32], f32, tag="pv01", name="pv01")
            pv23 = pv_pool.tile([P, 2, 8, 32], f32, tag="pv23", name="pv23")
            ms = ms_pool.tile([P, 512], f32, tag="ms", name="ms")
            ms_b = ms[:].bitcast(bf16)  # [P, 1024]: psT regions at 256/512

            qTh = [[consts.tile([P, 1024], bf16, tag=f"qT{g}{h}", name=f"qT{g}{h}") for h in range(2)] for g in range(2)]
            kTh = [[consts.tile([P, 1024], bf16, tag=f"kT{g}{h}", name=f"kT{g}{h}") for h in range(2)] for g in range(2)]
            vq = [consts.tile([P, 4, 8, 32], bf16, tag=f"vq{q}", name=f"vq{q}") for q in range(4)]

            def proj_qk_halves(w, g, half, c2):
                # one 512-wide chunk of the q or k projection for (g, half),
                # split into two 4-deep accumulation halves so filler pops
                # stay under the ACT step time
                lhs_sb = aq_sb if w == "q" else ak_sb
                qtr = 2 * half + c2
                state = {}

                def mm(ps, k):
                    nc.tensor.matmul(
                        ps[:],
                        lhsT=lhs_sb[:, k, 128 * g : 128 * g + 128],
                        rhs=xall[:, k, qtr],
                        start=(k == 0),
                        stop=(k == 7),
                        skip_group_check=True,
                    )

                def h0():
                    state["ps"] = ax_pool.tile([P, 512], f32, tag="ax", name="ax")
                    for k in range(4):
                        mm(state["ps"], k)

                def h1():
                    ps = state["ps"]
                    for k in range(4, 8):
                        mm(ps, k)
                    dst = (qTh if w == "q" else kTh)[g][half][:, 512 * c2 : 512 * c2 + 512]
                    if w == "q":
                        nc.vector.tensor_tensor(dst, ps[:], sll_sb[:, qtr], MULT)
                    else:
                        nc.scalar.copy(dst, ps[:])

                return [h0, h1]

            def proj_v_halves(st):
                qq, sti = st // 4, st % 4
                state = {}

                def mm(psv, k):
                    nc.tensor.matmul(
                        psv[:, :256],
                        lhsT=xall[:, k, qq, 128 * sti : 128 * sti + 128],
                        rhs=av_sb[:, k, :],
                        start=(k == 0),
                        stop=(k == 7),
                        skip_group_check=True,
                    )

                def h0():
                    state["ps"] = ax_pool.tile([P, 512], f32, tag="ax", name="ax")
                    for k in range(4):
                        mm(state["ps"], k)

                def h1():
                    psv = state["ps"]
                    for k in range(4, 8):
                        mm(psv, k)
                    if KNOBS["vq_alt"] and st % 2 == 1:
                        nc.vector.tensor_copy(
                            vq[qq][:, sti, :, :],
                            psv[:, :256].rearrange("p (h c) -> p h c", h=8),
                        )
                    else:
                        nc.scalar.copy(
                            vq[qq][:, sti, :, :],
                            psv[:, :256].rearrange("p (h c) -> p h c", h=8),
                        )

                return [h0, h1]

            def make_groups(r):
                # each group is a mutable list of halves sharing one ax tile;
                # halves of a group must emit with no other ax alloc between
                if r[0] == "qk":
                    _, half, c2 = r
                    return [
                        proj_qk_halves(w, g, half, c2)
                        for g in range(2)
                        for w in ("q", "k")
                    ]
                _, st = r
                return [proj_v_halves(st)]

            done: set = set()
            fillers: deque = deque()  # of group lists
            pending: deque = deque()  # forced-next second halves
            epi: deque = deque()
            group_of: dict = {}
            rr = [0]

            def emit_group_now(gr):
                while gr:
                    gr.pop(0)()

            def drain(r):
                # emit any not-yet-emitted units of resource r immediately
                while pending:
                    pending.popleft()()
                if r not in done:
                    done.add(r)
                    for gr in make_groups(r):
                        emit_group_now(gr)
                    return
                for gr in group_of.pop(r, []):
                    try:
                        fillers.remove(gr)
                    except ValueError:
                        pass  # already popped (possibly partially emitted)
                    emit_group_now(gr)

            def queue(r):
                if r in done:
                    return
                done.add(r)
                grs = make_groups(r)
                group_of[r] = grs
                fillers.extend(grs)

            def pop_work():
                if pending:
                    pending.popleft()()
                    return
                rr[0] += 1
                if rr[0] % 3 != 2 and epi:
                    epi.popleft()[1]()
                    return
                if fillers:
                    gr = fillers.popleft()
                    gr.pop(0)()
                    if gr:
                        pending.append(gr.pop(0))
                    return
                if epi:
                    epi.popleft()[1]()

            step_no = [0]
            drain_mode = [False]
            ob_ct = [0]

            def attention_qc(qc, is_last):
                entries = plans[qc]
                qh, qcol = qc // 2, 512 * (qc % 2)

                ats = [None] * 4
                aTs = [None] * 4

                def mk_divide(qt):
                    def _div():
                        rc = recip_pool.tile([P, 8], f32, tag="rc", name="rc")
                        nc.vector.reciprocal_approx_fast(
                            out=rc[:], in_=ms[:, 8 * qt : 8 * qt + 8]
                        )
                        at = att_pool.tile([P, 8, 32], bf16, tag="at", name="at")
                        nc.vector.tensor_tensor(
                            at[:],
                            (pv01 if qt < 2 else pv23)[:, qt % 2],
                            rc[:, :, None].to_broadcast((P, 8, 32)),
                            MULT,
                        )
                        ats[qt] = at[:]

                    return _div

                def mk_divide_pair(pair):
                    # one recip + one multiply for both qt of the pv bank:
                    # halves the DVE op count on the divide path
                    def _div():
                        rc = recip_pool.tile([P, 2, 8], f32, tag="rc2", name="rc2")
                        nc.vector.reciprocal_approx_fast(
                            out=rc[:], in_=ms[:, 16 * pair : 16 * pair + 16]
                        )
                        at = att_pool.tile([P, 2, 8, 32], bf16, tag="at2", name="at2")
                        nc.vector.tensor_tensor(
                            at[:],
                            (pv01 if pair == 0 else pv23)[:],
                            rc[:, :, :, None].to_broadcast((P, 2, 8, 32)),
                            MULT,
                        )
                        ats[2 * pair] = at[:, 0]
                        ats[2 * pair + 1] = at[:, 1]

                    return _div

                def mk_transpose(qt):
                    def _tr():
                        if drain_mode[0] and KNOBS["drain_sc"]:
                            axt = sc_pool.tile([P, 512], f32, tag="sc", name="sc")
                        else:
                            axt = ax_pool.tile([P, 512], f32, tag="ax", name="ax")
                        psT = axt[:].bitcast(bf16)[:, 0:256].rearrange("p (c q) -> p c q", c=2)
                        at2 = ats[qt].rearrange("p h c -> p (h c)")
                        for c in range(2):
                            nc.tensor.matmul(
                                psT[:, c],
                                lhsT=at2[:, 128 * c : 128 * c + 128],
                                rhs=id_sb[:],
                                is_transpose=True,
                            )
                        aT = attT_pool.tile([P, 2, 128], bf16, tag="aT", name="aT")
                        nc.vector.tensor_copy(aT[:], psT)
                        aTs[qt] = aT

                    return _tr

                def mk_wo(qt, n):
                    def _wo():
                        aT = aTs[qt]
                        st = 4 * qc + qt
                        if drain_mode[0] and KNOBS["drain_sc"]:
                            # final drain: scores are done, rotate wo psum
                            # through the free sc banks so consecutive wo
                            # matmuls don't serialize on the single ax bank
                            wops = sc_pool.tile([P, 512], f32, tag="sc", name="sc")
                        else:
                            wops = ax_pool.tile([P, 512], f32, tag="ax", name="ax")
                        for kk in range(2):
                            nc.tensor.matmul(
                                wops[:],
                                lhsT=aT[:, kk],
                                rhs=wor_sb[:, kk, 512 * n : 512 * n + 512],
                                start=(kk == 0),
                                stop=(kk == 1),
                            )
                        ob = oout_pool.tile([P, 512], bf16, tag="ob", name="ob")
                        ob_ct[0] += 1
                        if drain_mode[0] and KNOBS["ob_drain_alt"]:
                            # strict alternation: consecutive drain obs never
                            # queue behind each other on one engine
                            if ob_ct[0] % 2 == 0:
                                nc.vector.tensor_copy(ob[:], wops[:])
                            else:
                                nc.scalar.copy(ob[:], wops[:])
                            # spread DMA issue across two DGE queues
                            if ob_ct[0] % 2 == 0:
                                nc.scalar.dma_start(
                                    out=out[128 * st : 128 * st + 128, 512 * n : 512 * n + 512],
                                    in_=ob[:],
                                )
                            else:
                                nc.sync.dma_start(
                                    out=out[128 * st : 128 * st + 128, 512 * n : 512 * n + 512],
                                    in_=ob[:],
                                )
                            return
                        if KNOBS["ob_alt"] and n == 1:
                            nc.vector.tensor_copy(ob[:], wops[:])
                        else:
                            nc.scalar.copy(ob[:], wops[:])
                        nc.sync.dma_start(
                            out=out[128 * st : 128 * st + 128, 512 * n : 512 * n + 512],
                            in_=ob[:],
                        )

                    return _wo

                if not entries:
                    for qt in range(4):
                        at = att_pool.tile([P, 8, 32], bf16, tag="at", name="at")
                        nc.vector.memset(at[:], 0.0)
                        # transpose of zeros is zeros; emit wo directly on it
                        axt = ax_pool.tile([P, 512], f32, tag="ax", name="ax")
                        psT = axt[:].bitcast(bf16)[:, 0:256].rearrange("p (c q) -> p c q", c=2)
                        at2 = at[:].rearrange("p h c -> p (h c)")
                        for c in range(2):
                            nc.tensor.matmul(
                                psT[:, c],
                                lhsT=at2[:, 128 * c : 128 * c + 128],
                                rhs=id_sb[:],
                                is_transpose=True,
                            )
                        aT = attT_pool.tile([P, 2, 128], bf16, tag="aT", name="aT")
                        nc.vector.tensor_copy(aT[:], psT)
                        aTs[qt] = aT
                        epi.append(("wo", mk_wo(qt, 0)))
                        epi.append(("wo", mk_wo(qt, 1)))
                    return

                def pv_dn(g, jq, ei, kt, zq, pr):
                    # PV + denominator matmuls for an already-exp'd quarter.
                    # The first matmul into each bank this qc uses start=True
                    # (bank-wide pending-zero -> each region's first write
                    # overwrites); all later ones accumulate with start=False.
                    for jj in range(1):
                        h = 4 * g + jq
                        for qt in range(zq // P, 4):
                            lhsT = pr[:, 128 * qt : 128 * qt + 128]
                            pvb = "pv01" if qt < 2 else "pv23"
                            pvd = (pv01 if qt < 2 else pv23)[:, qt % 2, h, :]
                            nc.tensor.matmul(
                                pvd,
                                lhsT=lhsT,
                                rhs=vq[kt // 4][:, kt % 4, h, :],
                                start=pvb in fresh_banks,
                                stop=(ei == last_e[qc][qt]),
                                skip_group_check=True,
                            )
                            fresh_banks.discard(pvb)
                            nc.tensor.matmul(
                                ms[:, 8 * qt + h : 8 * qt + h + 1],
                                lhsT=lhsT,
                                rhs=ones_sb[:],
                                start="ms" in fresh_banks,
                                stop=(ei == last_e[qc][qt]),
                                skip_group_check=True,
                            )
                            fresh_banks.discard("ms")
                    if g == 1 and jq == 3:
                        if KNOBS["div_merge"]:
                            for pair in (0, 1):
                                qts = (2 * pair, 2 * pair + 1)
                                les = [last_e[qc][q] for q in qts
                                       if last_e[qc][q] is not None]
                                if les and max(les) == ei:
                                    epi.append(("div", mk_divide_pair(pair)))
                                    for q in qts:
                                        if last_e[qc][q] is not None:
                                            epi.append(("tr", mk_transpose(q)))
                                            epi.append(("wo", mk_wo(q, 0)))
                                            epi.append(("wo", mk_wo(q, 1)))
                        else:
                            for qt in range(4):
                                if ei == last_e[qc][qt]:
                                    epi.append(("div", mk_divide(qt)))
                                    epi.append(("tr", mk_transpose(qt)))
                                    epi.append(("wo", mk_wo(qt, 0)))
                                    epi.append(("wo", mk_wo(qt, 1)))

                # software-pipelined emission: scores+exp of step s+2 are
                # emitted BEFORE pv/dn of step s, so the in-order PE stream
                # never head-of-line blocks on the exp of the current entry.
                # quarter-entry software pipeline: each step covers ONE head
                # of one (g, entry) in ONE 1-bank [P,512] sc tile. sc bufs=4
                # gives a 4-deep rotation, so a late DVE-Schraudolph exp (its
                # queue holds other work) never stalls the ACT exp chain; the
                # exp engine is chosen per step (~1/3 DVE) to balance
                # makespans. Scores are emitted two steps ahead of their exp
                # so they precede the pv/filler bursts in the in-order PE
                # stream.
                hsteps = [
                    (g, jq, ei, kt, mi, zq, tri)
                    for ei, (kt, mi, zq, tri) in enumerate(entries)
                    for g in range(2)
                    for jq in range(4)
                ]

                def sc_emit(h):
                    g, jq, ei, kt, mi, zq, tri = h
                    kh, kcol = kt // 8, 128 * (kt % 8)
                    szq = zq  # bf16 matmul: no small-free-dim penalty
                    psc = sc_pool.tile([P, 512], f32, tag="sc", name="sc")
                    nc.tensor.matmul(
                        psc[:, szq:512],
                        lhsT=kTh[g][kh][32 * jq : 32 * jq + 32, kcol : kcol + 128],
                        rhs=qTh[g][qh][32 * jq : 32 * jq + 32, qcol + szq : qcol + 512],
                        start=True,
                        stop=True,
                        tile_position=(32 * jq, 0),
                    )
                    return psc

                def exp_emit(h, psc):
                    g, jq, ei, kt, mi, zq, tri = h
                    # scores pad the matmul to >=256 wide, but only [zq:] is
                    # ever read downstream -- exp just that
                    pr = probs_pool.tile([P, 512], bf16, tag="pr", name="pr")
                    ph = step_no[0] % KNOBS["exp_period"]
                    step_no[0] += 1
                    pq = KNOBS["dve_phases_qc"]
                    phases = pq[qc] if pq else KNOBS["dve_phases"]
                    use_dve = ph in phases
                    if use_dve:
                        nc.vector.tensor_scalar(
                            pr[:].bitcast(i16)[:, zq:], psc[:, zq:],
                            SCH_C1, SCH_C2, MULT, ADD,
                        )
                    else:
                        nc.scalar.activation(pr[:, zq:], psc[:, zq:], EXP)
                    if tri:
                        blk = pr[:, zq : zq + P]
                        te = KNOBS["tril_engine"]
                        if te == "follow":
                            te = "dve" if use_dve else "pool"
                        if te == "pool":
                            # SBUF-only bf16 multiply on idle GPSIMD
                            nc.gpsimd.tensor_tensor(blk, blk, tril_sb[:], MULT)
                        else:
                            nc.vector.tensor_tensor(blk, blk, tril_sb[:], MULT)
                    elif mi >= 0:
                        if em_res is not None:
                            emt = em_res[:, mi, :]
                        else:
                            emtt = emt_pool.tile([P, QC], bf16, tag="emt", name="emt")
                            nc.sync.dma_start(out=emtt[:], in_=emask[mi])
                            emt = emtt[:]
                        nc.vector.tensor_tensor(
                            pr[:, zq:], pr[:, zq:], emt[:, zq:], MULT
                        )
                    return pr

                if any(k == "div" for k, _ in epi):
                    rest = deque()
                    while epi:
                        k, fn = epi.popleft()
                        if k == "div":
                            fn()
                        else:
                            rest.append((k, fn))
                    epi.extend(rest)
                # no memsets: the chronologically first matmul into each
                # accumulator bank this qc carries start=True, which marks the
                # whole bank pending-zero -- every region's first write then
                # overwrites stale data (hw has_written semantics; the
                # interpreter models the same bank-granular pending-zero)
                fresh_banks = {"pv01", "pv23", "ms"}

                ns = len(hsteps)
                pscs: dict = {}
                prs: dict = {}
                for s in range(ns):
                    if s == 0:
                        pscs[0] = sc_emit(hsteps[0])
                        if ns > 1:
                            pscs[1] = sc_emit(hsteps[1])
                    if s + 2 < ns:
                        pscs[s + 2] = sc_emit(hsteps[s + 2])
                    prs[s] = exp_emit(hsteps[s], pscs.pop(s))
                    lag = KNOBS["pv_lag"]
                    if s >= lag:
                        h = hsteps[s - lag]
                        drain(("v", h[3]))
                        pv_dn(h[0], h[1], h[2], h[3], h[5], prs.pop(s - lag))
                    if (is_last and KNOBS["aggr_last"]) or s % 2 == 0 or len(epi) + len(fillers) > 8:
                        pop_work()
                for s in range(max(0, ns - KNOBS["pv_lag"]), ns):
                    if s < 0 or s not in prs:
                        continue
                    h = hsteps[s]
                    drain(("v", h[3]))
                    pv_dn(h[0], h[1], h[2], h[3], h[5], prs.pop(s))
                    pop_work()

            def needs(qc):
                res = [("qk", qc // 2, qc % 2)]
                for kt, _, _, _ in plans[qc]:
                    r = ("qk", kt // 8, (kt % 8) // 4)
                    if r not in res:
                        res.append(r)
                return res

            qorder = list(KNOBS["qc_order"])
            for qi, qc in enumerate(qorder):
                for r in needs(qc):
                    drain(r)
                for kt, _, _, _ in plans[qc]:
                    queue(("v", kt))
                if qi + 1 < len(qorder):
                    nqc2 = qorder[qi + 1]
                    for r in needs(nqc2):
                        queue(r)
                    for kt, _, _, _ in plans[nqc2]:
                        queue(("v", kt))
                attention_qc(qc, qi == len(qorder) - 1)
            # final drain: divides/transposes first so the wo chains overlap;
            # wo psum rotates through the now-idle sc banks
            drain_mode[0] = True
            _order = {"div": 0, "tr": 1}
            _rest = sorted(epi, key=lambda kv: _order.get(kv[0], 2))
            epi.clear()
            epi.extend(_rest)
            while epi or fillers:
                pop_work()

    if not nc.is_finalized():
        nc.finalize()
    return nc


def _round_f32r(a):
    """Round fp32 array to the PE's f32r format (mantissa truncated to 11
    bits, round-to-nearest-even at bit 12) so f32r-declared DMA inputs match
    what an on-device cast would produce."""
    u = np.ascontiguousarray(a, dtype=np.float32).view(np.uint32)
    u2 = (u + np.uint32(0x7FF) + ((u >> np.uint32(12)) & np.uint32(1))) & np.uint32(0xFFFFF000)
    return u2.view(np.float32)


def _plan_from_mask(mask):
    """Classify [128, 512] tiles of exp(mask)^T; returns (plans, packed_tiles).

    Entries are (kt, mi, zq, tri): zq (multiple of 128) leading fully-masked
    q-columns; tri=True means the tile is [zeros | tril(128) | ones] so only
    the shared tril block needs multiplying; mi >= 0 indexes a packed general
    bf16 exp(mask) tile.
    """
    em = np.exp(mask.astype(np.float32))  # [q, k]
    emT = np.ascontiguousarray(em.T)  # [k, q]
    # partial diagonal block in [k, q] layout: valid iff q_local >= k_local
    tril_blk = np.triu(np.ones((P, P), dtype=np.float32))
    plans = []
    tiles = []
    tile_keys = {}
    for qc in range(NQC):
        ent = []
        covered = [False] * 4
        for kt in range(NKT):
            t = emT[P * kt : P * (kt + 1), QC * qc : QC * (qc + 1)]
            if not t.any():
                continue  # fully masked out: skip tile entirely
            if (t == 1.0).all():
                ent.append((kt, -1, 0, False))
                continue
            nz = np.flatnonzero(t.any(axis=0))
            zq = (int(nz[0]) // P) * P
            # tril-structured tile: [zeros(zq) | tril | ones]
            tri = (
                zq + P <= QC
                and (t[:, :zq] == 0.0).all()
                and (t[:, zq : zq + P] == tril_blk).all()
                and (t[:, zq + P :] == 1.0).all()
            )
            if tri:
                ent.append((kt, -1, zq, True))
                continue
            key = t.tobytes()
            mi = tile_keys.get(key)
            if mi is None:
                mi = len(tiles)
                tile_keys[key] = mi
                tiles.append(t.astype(ml_dtypes.bfloat16))
            ent.append((kt, mi, zq, False))
        for kt, mi, zq, tri in ent:
            for qt in range(zq // P, 4):
                covered[qt] = True
        if ent and not all(covered):
            # some qt block would never be written: disable skipping (the
            # emask multiply zeroes masked probs so pv/dn stay correct)
            ent2 = []
            for kt, mi, zq, tri in ent:
                if zq == 0:
                    ent2.append((kt, mi, zq, tri))
                    continue
                t = emT[P * kt : P * (kt + 1), QC * qc : QC * (qc + 1)]
                key = t.tobytes()
                mi = tile_keys.get(key)
                if mi is None:
                    mi = len(tiles)
                    tile_keys[key] = mi
                    tiles.append(t.astype(ml_dtypes.bfloat16))
                ent2.append((kt, mi, 0, False))
            ent = ent2
        plans.append(tuple(ent))
    if tiles:
        packed = np.ascontiguousarray(np.stack(tiles))
    else:
        packed = np.zeros((1, P, QC), dtype=ml_dtypes.bfloat16)
    return tuple(plans), packed


def kernel(x, mask, section_log_len, wq, wk, wv, wo, seq_scale):
    from concourse.bass_utils import run_bass_kernel_spmd

    x = np.asarray(x, dtype=np.float32)
    assert x.shape == (B, S, D), x.shape
    mask2 = np.asarray(mask, dtype=np.float32).reshape(S, S)
    sll = np.asarray(section_log_len, dtype=np.float32).reshape(S)
    ss = np.asarray(seq_scale, dtype=np.float32).reshape(H)
    wq = np.asarray(wq, dtype=np.float32)
    wk = np.asarray(wk, dtype=np.float32)
    wv = np.asarray(wv, dtype=np.float32)
    wo = np.asarray(wo, dtype=np.float32)

    plans, tiles = _plan_from_mask(mask2)
    key = (plans, tiles.shape[0])
    nc = _GRAPH_CACHE.get(key)
    if nc is None:
        nc = _build_graph(plans, tiles.shape[0])
        _GRAPH_CACHE[key] = nc

    bf = ml_dtypes.bfloat16
    sllB = np.ascontiguousarray(
        np.broadcast_to(sll[None, :], (P, S)), dtype=np.float32
    )
    xT = [np.ascontiguousarray(x[b].T).astype(bf) for b in range(B)]
    trilB = np.triu(np.ones((P, P), dtype=np.float32)).astype(bf)
    identB = np.eye(P, dtype=np.float32).astype(bf)

    in_maps = []
    for c in range(NCORES):
        b, g2 = divmod(c, 4)
        rows = slice(256 * g2, 256 * (g2 + 1))
        ssr = np.repeat(ss[8 * g2 : 8 * g2 + 8], HD) / math.sqrt(HD)
        in_maps.append(
            {
                "xT": xT[b],
                "aq": np.ascontiguousarray((wq[rows, :] * ssr[:, None]).T).astype(bf),
                "ak": np.ascontiguousarray(wk[rows, :].T).astype(bf),
                "av": np.ascontiguousarray(wv[rows, :].T).astype(bf),
                "wor": np.ascontiguousarray(wo[:, rows].T).astype(bf),
                "sllb": sllB,
                "tril": trilB,
                "ident": identB,
                "emask": tiles,
            }
        )

    res = run_bass_kernel_spmd(nc, in_maps, core_ids=list(range(NCORES))).results
    out = np.zeros((B, S, D), dtype=np.float32)
    for c in range(NCORES):
        out[c // 4] += np.asarray(res[c]["out"], dtype=np.float32)
    return out



# revision 30
# speedup vs baseline: 1.0697x; 1.0243x over previous
"""Trainium2 Bass kernel for nn_Attention_80917183857290.

Multi-head causal attention (B=2, S=2048, D=1024, H=32, HD=32) with
SSMax-style per-query log-length score scaling, run SPMD on 8 NeuronCores.

Sharding: core c -> batch b = c // 4, head-group g2 = c % 4 (8 heads each).
Per core:
  - projections q,k (transposed layout [head_dim rows, seq]) and v, f32r
    (TF32-like single-pass fp32); x and sll stream in seq-quarters so the
    first attention chunk starts ~15us in; PE-clock warmup dummies burn the
    p-state ramp during the initial DMA.
  - scoresT [128k, 512q] per head per k-tile (f32r, K=32 via tile_position
    quads); a quarter-entry software pipeline (one 1-bank PSUM tile per
    step, 4-deep rotation, scores emitted 2 steps ahead) keeps the
    scores -> exp -> PV chain free of in-order head-of-line stalls.
  - probs = exp(scores * sll * ss / sqrt(hd)): ~4/7 on ACT, ~3/7 on DVE via
    a bf16 Schraudolph bit-trick (tensor_scalar -> int16, bitcast bf16,
    ~2% elementwise error that mostly cancels through softmax).
  - PV with probs as the stationary operand: out [128q, 32hd] per (head,
    kt) accumulated in PSUM across kt -- the narrow free dim makes PV ~4x
    cheaper on the PE than the scoresT-layout PV. Denominators via
    per-head [128q, 1] matmuls against a ones column. The multi-region
    accumulator banks are memset once per qc and accumulated with
    start=False: hardware start=True resets accumulation state at BANK
    granularity and would clobber sibling regions.
  - att = pv * recip(dn) (bf16); att^T via PE transpose (identity matmul);
    out = att^T.T @ wo_shard (f32r); projections/epilogue work is woven
    into the attention steps as paced filler units.
  - host sums the 4 partial outputs per batch.

The causal mask reduces to a single shared [128,128] triangular tile
multiplied only on diagonal-crossing blocks; fully-masked [128 k, 128 q]
blocks are skipped in scores/exp/PV entirely. Non-causal masks fall back
to per-tile bf16 exp(mask) multiplies (correct for any mask).
"""

import math

import numpy as np
import ml_dtypes

B, S, D, H = 2, 2048, 1024, 32
HD = D // H  # 32
P = 128
QC = 512  # q-chunk (PSUM bank free size, fp32)
NQC = S // QC  # 4
NKT = S // P  # 16
NCORES = 8

_GRAPH_CACHE: dict = {}


# exp engine schedule: step_no cycles through EXP_PERIOD phases; phases in
# DVE_PHASES run the Schraudolph exp on DVE, the rest true-exp on ACT.
# GPSIMD (Pool) has no PSUM port, so it cannot read scores; it instead
# absorbs the SBUF-only tril multiplies. Ratio balances ACT/DVE makespans
# (ACT 612ns/step + copies vs DVE 658ns/step + TT/memset work).
EXP_PERIOD = 17
DVE_PHASES = (0, 2, 4, 6, 8, 10, 12, 14, 16)

# scheduling knobs (A/B-tunable from bench.py)
KNOBS = {
    "tril_engine": "follow",   # pool | dve | follow (dve after dve-exp, pool after act-exp)
    "interleave": False,       # spread diag entries among full entries
    "pv_lag": 3,              # steps between exp and pv consumption
    "aggr_last": True,        # pop_work every step in the last qc
    "dummies": 9,             # warmup matmul count
    "x_split": 3,             # DMAs per x quarter
    "drain_sc": True,         # wo/tr psum from sc banks during final drain
    "ob_drain_alt": True,     # alternate ob copy engine in final drain
    "exp_period": 15,
    "dve_phases": (1, 3, 5, 7, 9, 11, 13, 14),
    "dve_phases_qc": None,     # optional per-qc override: tuple of 4 phase-tuples
    "qc_order": (0, 1, 2, 3),  # processing order of q-chunks
    "ob_alt": False,           # alternate ob copy engine globally
    "vq_alt": False,           # alternate vq copy engine
    "div_merge": False,         # one divide per qt-pair instead of per qt
}
# bf16 Schraudolph constants: bits = round(x * 128/ln2 + (127*128 - 7.4))
SCH_C1 = 128.0 / math.log(2.0)
SCH_C2 = 127.0 * 128.0 - 7.4


def _build_graph(plans, nt, reps=1):
    """Build the per-core Bass graph.

    plans: tuple over qc (4) of tuple of (kt, mi, zq, tri) entries; mi -1
      means no general-mask multiply; >=0 indexes the packed emask tiles;
      tri means multiply the shared [128,128] triu tile on block zq//128.
      zq is a multiple of 128: leading zq q-columns are fully masked.
    nt: number of packed [128, 512] bf16 exp-mask tiles (>= 1).
    """
    from collections import deque

    import concourse.mybir as mybir
    from concourse import bacc
    from concourse.tile import TileContext

    f32 = mybir.dt.float32
    f32r = mybir.dt.float32r
    bf16 = mybir.dt.bfloat16
    i16 = mybir.dt.int16
    EXP = mybir.ActivationFunctionType.Exp
    MULT = mybir.AluOpType.mult
    ADD = mybir.AluOpType.add

    nc = bacc.Bacc()

    xT = nc.declare_dram_parameter("xT", [D, S], bf16, isOutput=False)
    aq = nc.declare_dram_parameter("aq", [D, 256], bf16, isOutput=False)
    ak = nc.declare_dram_parameter("ak", [D, 256], bf16, isOutput=False)
    av = nc.declare_dram_parameter("av", [D, 256], bf16, isOutput=False)
    wor = nc.declare_dram_parameter("wor", [256, D], bf16, isOutput=False)
    sllb = nc.declare_dram_parameter("sllb", [P, S], f32, isOutput=False)
    tril = nc.declare_dram_parameter("tril", [P, P], bf16, isOutput=False)
    ident = nc.declare_dram_parameter("ident", [P, P], bf16, isOutput=False)
    emask = nc.declare_dram_parameter("emask", [nt, P, QC], bf16, isOutput=False)
    out = nc.declare_dram_parameter("out", [S, D], bf16, isOutput=True)

    # spread "special" entries (tril / masked / zq-skip) evenly among the
    # full-tile entries: their post-exp Pool/DVE multiplies otherwise cluster
    # at the end of each qc and stall the PV chain on the Pool queue
    def _interleave(ent):
        fulls = [e for e in ent if not e[3] and e[1] < 0 and e[2] == 0]
        specials = [e for e in ent if e[3] or e[1] >= 0 or e[2] > 0]
        if not fulls or not specials:
            return tuple(ent)
        n = len(fulls) + len(specials)
        res, fi, si = [], 0, 0
        for i in range(n):
            # Bresenham-style even spread of specials
            if si * n <= i * len(specials) and si < len(specials):
                res.append(specials[si]); si += 1
            elif fi < len(fulls):
                res.append(fulls[fi]); fi += 1
            else:
                res.append(specials[si]); si += 1
        return tuple(res)

    if KNOBS["interleave"]:
        plans = tuple(_interleave(p) for p in plans)

    any_general = any(mi >= 0 for p in plans for (_, mi, _, _) in p)

    # first/last contributing entry index per (qc, qt) for pv/dn accumulation
    first_e = [[None] * 4 for _ in range(NQC)]
    last_e = [[None] * 4 for _ in range(NQC)]
    for qc in range(NQC):
        for ei, (kt, mi, zq, tri) in enumerate(plans[qc]):
            for qt in range(zq // P, 4):
                if first_e[qc][qt] is None:
                    first_e[qc][qt] = ei
                last_e[qc][qt] = ei

    with TileContext(nc) as tc:
        with (
            tc.tile_pool(name="consts", bufs=1) as consts,
            tc.tile_pool(name="sc", bufs=4, space="PSUM") as sc_pool,
            tc.tile_pool(name="pv", bufs=1, space="PSUM") as pv_pool,
            tc.tile_pool(name="ms", bufs=1, space="PSUM") as ms_pool,
            tc.tile_pool(name="ax", bufs=1, space="PSUM") as ax_pool,
            tc.tile_pool(name="probs", bufs=8) as probs_pool,
            tc.tile_pool(name="emt", bufs=4) as emt_pool,
            tc.tile_pool(name="att", bufs=4) as att_pool,
            tc.tile_pool(name="attT", bufs=2) as attT_pool,
            tc.tile_pool(name="oout", bufs=4) as oout_pool,
            tc.tile_pool(name="recip", bufs=4) as recip_pool,
        ):
          for _rep in range(reps):
            # ---- constant loads, ordered by first use; xT and sll stream in
            # seq-quarters so the first attention chunk starts ~15us in ----
            # PE p-state warmup: dummy wide matmuls (no DMA deps) burn the
            # clock-ramp window while the first input DMAs are in flight
            ones_sb = consts.tile([P, 1], bf16, tag="ones", name="ones")
            nc.vector.memset(ones_sb[:], 1.0)
            dumm = consts.tile([P, 512], bf16, tag="dumm", name="dumm")
            nc.vector.memset(dumm[:], 0.0)
            # force the Exp table load onto ACT now, so it overlaps the input
            # DMA window instead of stalling the first k-proj copy
            actw = consts.tile([P, 1], f32, tag="actw", name="actw")
            nc.scalar.activation(actw[:], ones_sb[:], EXP)
            wps = ax_pool.tile([P, 512], f32, tag="ax", name="ax")
            for _ in range(KNOBS["dummies"]):
                nc.tensor.matmul(
                    wps[0:1, :],
                    lhsT=ones_sb[:],
                    rhs=dumm[:],
                    start=True,
                    stop=True,
                    skip_group_check=True,
                )

            aq_sb = consts.tile([P, 8, 256], bf16, tag="aq", name="aq")
            aq_r = aq.rearrange("(ko ki) f -> ki ko f", ki=P)
            nc.sync.dma_start(out=aq_sb[:, :, 0:128], in_=aq_r[:, :, 0:128])

            sll_sb = consts.tile([P, 4, QC], f32, tag="sll", name="sll")
            xall = consts.tile([P, 8, 4, QC], bf16, tag="xall", name="xall")
            xT_r = xT.rearrange("(ko ki) f -> ki ko f", ki=P)

            def load_quarter(qtr):
                # batched DMA per 8/xs contraction chunks: proj h0 (k 0-3)
                # can start before later chunks land
                xs = KNOBS["x_split"]
                for j in range(xs):
                    k0, k1 = 8 * j // xs, 8 * (j + 1) // xs
                    nc.sync.dma_start(
                        out=xall[:, k0:k1, qtr],
                        in_=xT_r[:, k0:k1, QC * qtr : QC * (qtr + 1)],
                    )
                    if j == 0:
                        # sll gates the q-proj h1 multiply: land it before
                        # the second x half
                        nc.sync.dma_start(
                            out=sll_sb[:, qtr],
                            in_=sllb[:, QC * qtr : QC * (qtr + 1)],
                        )

            load_quarter(0)
            ak_sb = consts.tile([P, 8, 256], bf16, tag="ak", name="ak")
            ak_r = ak.rearrange("(ko ki) f -> ki ko f", ki=P)
            nc.sync.dma_start(out=ak_sb[:, :, 0:128], in_=ak_r[:, :, 0:128])
            nc.sync.dma_start(out=aq_sb[:, :, 128:256], in_=aq_r[:, :, 128:256])
            nc.sync.dma_start(out=ak_sb[:, :, 128:256], in_=ak_r[:, :, 128:256])
            tril_sb = consts.tile([P, P], bf16, tag="tril", name="tril")
            nc.sync.dma_start(out=tril_sb[:], in_=tril[:])
            em_res = None
            if any_general and nt <= 16:
                em_res = consts.tile([P, nt, QC], bf16, tag="emres", name="emres")
                nc.sync.dma_start(out=em_res[:], in_=emask.rearrange("t p f -> p t f"))
            av_sb = consts.tile([P, 8, 256], bf16, tag="av", name="av")
            nc.sync.dma_start(out=av_sb[:], in_=av.rearrange("(ko ki) f -> ki ko f", ki=P))
            load_quarter(1)
            id_sb = consts.tile([P, P], bf16, tag="ident", name="ident")
            nc.sync.dma_start(out=id_sb[:], in_=ident[:])
            wor_sb = consts.tile([P, 2, D], bf16, tag="wor", name="wor")
            nc.sync.dma_start(out=wor_sb[:], in_=wor.rearrange("(ko ki) f -> ki ko f", ki=P))
            load_quarter(2)
            load_quarter(3)

            # persistent PSUM accumulators (slice-granular reuse across qc)
            pv01 = pv_pool.tile([P, 2, 8, 32], f32, tag="pv01", name="pv01")
            pv23 = pv_pool.tile([P, 2, 8, 32], f32, tag="pv23", name="pv23")
            ms = ms_pool.tile([P, 512], f32, tag="ms", name="ms")
            ms_b = ms[:].bitcast(bf16)  # [P, 1024]: psT regions at 256/512

            qTh = [[consts.tile([P, 1024], bf16, tag=f"qT{g}{h}", name=f"qT{g}{h}") for h in range(2)] for g in range(2)]
            kTh = [[consts.tile([P, 1024], bf16, tag=f"kT{g}{h}", name=f"kT{g}{h}") for h in range(2)] for g in range(2)]
            vq = [consts.tile([P, 4, 8, 32], bf16, tag=f"vq{q}", name=f"vq{q}") for q in range(4)]

            def proj_qk_halves(w, g, half, c2):
                # one 512-wide chunk of the q or k projection for (g, half),
                # split into two 4-deep accumulation halves so filler pops
                # stay under the ACT step time
                lhs_sb = aq_sb if w == "q" else ak_sb
                qtr = 2 * half + c2
                state = {}

                def mm(ps, k):
                    nc.tensor.matmul(
                        ps[:],
                        lhsT=lhs_sb[:, k, 128 * g : 128 * g + 128],
                        rhs=xall[:, k, qtr],
                        start=(k == 0),
                        stop=(k == 7),
                        skip_group_check=True,
                    )

                def h0():
                    state["ps"] = ax_pool.tile([P, 512], f32, tag="ax", name="ax")
                    for k in range(4):
                        mm(state["ps"], k)

                def h1():
                    ps = state["ps"]
                    for k in range(4, 8):
                        mm(ps, k)
                    dst = (qTh if w == "q" else kTh)[g][half][:, 512 * c2 : 512 * c2 + 512]
                    if w == "q":
                        nc.vector.tensor_tensor(dst, ps[:], sll_sb[:, qtr], MULT)
                    else:
                        nc.scalar.copy(dst, ps[:])

                return [h0, h1]

            def proj_v_halves(st):
                qq, sti = st // 4, st % 4
                state = {}

                def mm(psv, k):
                    nc.tensor.matmul(
                        psv[:, :256],
                        lhsT=xall[:, k, qq, 128 * sti : 128 * sti + 128],
                        rhs=av_sb[:, k, :],
                        start=(k == 0),
                        stop=(k == 7),
                        skip_group_check=True,
                    )

                def h0():
                    state["ps"] = ax_pool.tile([P, 512], f32, tag="ax", name="ax")
                    for k in range(4):
                        mm(state["ps"], k)

                def h1():
                    psv = state["ps"]
                    for k in range(4, 8):
                        mm(psv, k)
                    if KNOBS["vq_alt"] and st % 2 == 1:
                        nc.vector.tensor_copy(
                            vq[qq][:, sti, :, :],
                            psv[:, :256].rearrange("p (h c) -> p h c", h=8),
                        )
                    else:
                        nc.scalar.copy(
                            vq[qq][:, sti, :, :],
                            psv[:, :256].rearrange("p (h c) -> p h c", h=8),
                        )

                return [h0, h1]

            def make_groups(r):
                # each group is a mutable list of halves sharing one ax tile;
                # halves of a group must emit with no other ax alloc between
                if r[0] == "qk":
                    _, half, c2 = r
                    return [
                        proj_qk_halves(w, g, half, c2)
                        for g in range(2)
                        for w in ("q", "k")
                    ]
                _, st = r
                return [proj_v_halves(st)]

            done: set = set()
            fillers: deque = deque()  # of group lists
            pending: deque = deque()  # forced-next second halves
            epi: deque = deque()
            group_of: dict = {}
            rr = [0]

            def emit_group_now(gr):
                while gr:
                    gr.pop(0)()

            def drain(r):
                # emit any not-yet-emitted units of resource r immediately
                while pending:
                    pending.popleft()()
                if r not in done:
                    done.add(r)
                    for gr in make_groups(r):
                        emit_group_now(gr)
                    return
                for gr in group_of.pop(r, []):
                    try:
                        fillers.remove(gr)
                    except ValueError:
                        pass  # already popped (possibly partially emitted)
                    emit_group_now(gr)

            def queue(r):
                if r in done:
                    return
                done.add(r)
                grs = make_groups(r)
                group_of[r] = grs
                fillers.extend(grs)

            def pop_work():
                if pending:
                    pending.popleft()()
                    return
                rr[0] += 1
                if rr[0] % 3 != 2 and epi:
                    epi.popleft()[1]()
                    return
                if fillers:
                    gr = fillers.popleft()
                    gr.pop(0)()
                    if gr:
                        pending.append(gr.pop(0))
                    return
                if epi:
                    epi.popleft()[1]()

            step_no = [0]
            drain_mode = [False]
            ob_ct = [0]

            def attention_qc(qc, is_last):
                entries = plans[qc]
                qh, qcol = qc // 2, 512 * (qc % 2)

                ats = [None] * 4
                aTs = [None] * 4

                def mk_divide(qt):
                    def _div():
                        rc = recip_pool.tile([P, 8], f32, tag="rc", name="rc")
                        nc.vector.reciprocal_approx_fast(
                            out=rc[:], in_=ms[:, 8 * qt : 8 * qt + 8]
                        )
                        at = att_pool.tile([P, 8, 32], bf16, tag="at", name="at")
                        nc.vector.tensor_tensor(
                            at[:],
                            (pv01 if qt < 2 else pv23)[:, qt % 2],
                            rc[:, :, None].to_broadcast((P, 8, 32)),
                            MULT,
                        )
                        ats[qt] = at[:]

                    return _div

                def mk_divide_pair(pair):
                    # one recip + one multiply for both qt of the pv bank:
                    # halves the DVE op count on the divide path
                    def _div():
                        rc = recip_pool.tile([P, 2, 8], f32, tag="rc2", name="rc2")
                        nc.vector.reciprocal_approx_fast(
                            out=rc[:], in_=ms[:, 16 * pair : 16 * pair + 16]
                        )
                        at = att_pool.tile([P, 2, 8, 32], bf16, tag="at2", name="at2")
                        nc.vector.tensor_tensor(
                            at[:],
                            (pv01 if pair == 0 else pv23)[:],
                            rc[:, :, :, None].to_broadcast((P, 2, 8, 32)),
                            MULT,
                        )
                        ats[2 * pair] = at[:, 0]
                        ats[2 * pair + 1] = at[:, 1]

                    return _div

                def mk_transpose(qt):
                    def _tr():
                        if drain_mode[0] and KNOBS["drain_sc"]:
                            axt = sc_pool.tile([P, 512], f32, tag="sc", name="sc")
                        else:
                            axt = ax_pool.tile([P, 512], f32, tag="ax", name="ax")
                        psT = axt[:].bitcast(bf16)[:, 0:256].rearrange("p (c q) -> p c q", c=2)
                        at2 = ats[qt].rearrange("p h c -> p (h c)")
                        for c in range(2):
                            nc.tensor.matmul(
                                psT[:, c],
                                lhsT=at2[:, 128 * c : 128 * c + 128],
                                rhs=id_sb[:],
                                is_transpose=True,
                            )
                        aT = attT_pool.tile([P, 2, 128], bf16, tag="aT", name="aT")
                        nc.vector.tensor_copy(aT[:], psT)
                        aTs[qt] = aT

                    return _tr

                ob2s = {}

                def mk_wo(qt, n):
                    def _wo():
                        aT = aTs[qt]
                        st = 4 * qc + qt
                        if drain_mode[0] and KNOBS["drain_sc"]:
                            # final drain: scores are done, rotate wo psum
                            # through the free sc banks so consecutive wo
                            # matmuls don't serialize on the single ax bank
                            wops = sc_pool.tile([P, 512], f32, tag="sc", name="sc")
                        else:
                            wops = ax_pool.tile([P, 512], f32, tag="ax", name="ax")
                        for kk in range(2):
                            nc.tensor.matmul(
                                wops[:],
                                lhsT=aT[:, kk],
                                rhs=wor_sb[:, kk, 512 * n : 512 * n + 512],
                                start=(kk == 0),
                                stop=(kk == 1),
                            )
                        # both 512-halves share one [P, 1024] ob tile and a
                        # single out-DMA: halves the HWDGE issue count
                        if n == 0:
                            ob2s[st] = oout_pool.tile([P, 2, 512], bf16, tag="ob", name="ob")
                        ob = ob2s[st]
                        ob_ct[0] += 1
                        use_dve = (drain_mode[0] and KNOBS["ob_drain_alt"]
                                   and ob_ct[0] % 2 == 0)
                        if use_dve:
                            nc.vector.tensor_copy(ob[:, n], wops[:])
                        else:
                            nc.scalar.copy(ob[:, n], wops[:])
                        if n == 1:
                            del ob2s[st]
                            nc.sync.dma_start(
                                out=out[128 * st : 128 * st + 128, :],
                                in_=ob[:].rearrange("p c f -> p (c f)"),
                            )

                    return _wo

                if not entries:
                    for qt in range(4):
                        at = att_pool.tile([P, 8, 32], bf16, tag="at", name="at")
                        nc.vector.memset(at[:], 0.0)
                        # transpose of zeros is zeros; emit wo directly on it
                        axt = ax_pool.tile([P, 512], f32, tag="ax", name="ax")
                        psT = axt[:].bitcast(bf16)[:, 0:256].rearrange("p (c q) -> p c q", c=2)
                        at2 = at[:].rearrange("p h c -> p (h c)")
                        for c in range(2):
                            nc.tensor.matmul(
                                psT[:, c],
                                lhsT=at2[:, 128 * c : 128 * c + 128],
                                rhs=id_sb[:],
                                is_transpose=True,
                            )
                        aT = attT_pool.tile([P, 2, 128], bf16, tag="aT", name="aT")
                        nc.vector.tensor_copy(aT[:], psT)
                        aTs[qt] = aT
                        epi.append(("wo", mk_wo(qt, 0)))
                        epi.append(("wo", mk_wo(qt, 1)))
                    return

                def pv_dn(g, jq, ei, kt, zq, pr):
                    # PV + denominator matmuls for an already-exp'd quarter.
                    # The first matmul into each bank this qc uses start=True
                    # (bank-wide pending-zero -> each region's first write
                    # overwrites); all later ones accumulate with start=False.
                    for jj in range(1):
                        h = 4 * g + jq
                        for qt in range(zq // P, 4):
                            lhsT = pr[:, 128 * qt : 128 * qt + 128]
                            pvb = "pv01" if qt < 2 else "pv23"
                            pvd = (pv01 if qt < 2 else pv23)[:, qt % 2, h, :]
                            nc.tensor.matmul(
                                pvd,
                                lhsT=lhsT,
                                rhs=vq[kt // 4][:, kt % 4, h, :],
                                start=pvb in fresh_banks,
                                stop=(ei == last_e[qc][qt]),
                                skip_group_check=True,
                            )
                            fresh_banks.discard(pvb)
                            nc.tensor.matmul(
                                ms[:, 8 * qt + h : 8 * qt + h + 1],
                                lhsT=lhsT,
                                rhs=ones_sb[:],
                                start="ms" in fresh_banks,
                                stop=(ei == last_e[qc][qt]),
                                skip_group_check=True,
                            )
                            fresh_banks.discard("ms")
                    if g == 1 and jq == 3:
                        if KNOBS["div_merge"]:
                            for pair in (0, 1):
                                qts = (2 * pair, 2 * pair + 1)
                                les = [last_e[qc][q] for q in qts
                                       if last_e[qc][q] is not None]
                                if les and max(les) == ei:
                                    epi.append(("div", mk_divide_pair(pair)))
                                    for q in qts:
                                        if last_e[qc][q] is not None:
                                            epi.append(("tr", mk_transpose(q)))
                                            epi.append(("wo", mk_wo(q, 0)))
                                            epi.append(("wo", mk_wo(q, 1)))
                        else:
                            for qt in range(4):
                                if ei == last_e[qc][qt]:
                                    epi.append(("div", mk_divide(qt)))
                                    epi.append(("tr", mk_transpose(qt)))
                                    epi.append(("wo", mk_wo(qt, 0)))
                                    epi.append(("wo", mk_wo(qt, 1)))

                # software-pipelined emission: scores+exp of step s+2 are
                # emitted BEFORE pv/dn of step s, so the in-order PE stream
                # never head-of-line blocks on the exp of the current entry.
                # quarter-entry software pipeline: each step covers ONE head
                # of one (g, entry) in ONE 1-bank [P,512] sc tile. sc bufs=4
                # gives a 4-deep rotation, so a late DVE-Schraudolph exp (its
                # queue holds other work) never stalls the ACT exp chain; the
                # exp engine is chosen per step (~1/3 DVE) to balance
                # makespans. Scores are emitted two steps ahead of their exp
                # so they precede the pv/filler bursts in the in-order PE
                # stream.
                hsteps = [
                    (g, jq, ei, kt, mi, zq, tri)
                    for ei, (kt, mi, zq, tri) in enumerate(entries)
                    for g in range(2)
                    for jq in range(4)
                ]

                def sc_emit(h):
                    g, jq, ei, kt, mi, zq, tri = h
                    kh, kcol = kt // 8, 128 * (kt % 8)
                    szq = zq  # bf16 matmul: no small-free-dim penalty
                    psc = sc_pool.tile([P, 512], f32, tag="sc", name="sc")
                    nc.tensor.matmul(
                        psc[:, szq:512],
                        lhsT=kTh[g][kh][32 * jq : 32 * jq + 32, kcol : kcol + 128],
                        rhs=qTh[g][qh][32 * jq : 32 * jq + 32, qcol + szq : qcol + 512],
                        start=True,
                        stop=True,
                        tile_position=(32 * jq, 0),
                    )
                    return psc

                def exp_emit(h, psc):
                    g, jq, ei, kt, mi, zq, tri = h
                    # scores pad the matmul to >=256 wide, but only [zq:] is
                    # ever read downstream -- exp just that
                    pr = probs_pool.tile([P, 512], bf16, tag="pr", name="pr")
                    ph = step_no[0] % KNOBS["exp_period"]
                    step_no[0] += 1
                    pq = KNOBS["dve_phases_qc"]
                    phases = pq[qc] if pq else KNOBS["dve_phases"]
                    use_dve = ph in phases
                    if use_dve:
                        nc.vector.tensor_scalar(
                            pr[:].bitcast(i16)[:, zq:], psc[:, zq:],
                            SCH_C1, SCH_C2, MULT, ADD,
                        )
                    else:
                        nc.scalar.activation(pr[:, zq:], psc[:, zq:], EXP)
                    if tri:
                        blk = pr[:, zq : zq + P]
                        te = KNOBS["tril_engine"]
                        if te == "follow":
                            te = "dve" if use_dve else "pool"
                        if te == "pool":
                            # SBUF-only bf16 multiply on idle GPSIMD
                            nc.gpsimd.tensor_tensor(blk, blk, tril_sb[:], MULT)
                        else:
                            nc.vector.tensor_tensor(blk, blk, tril_sb[:], MULT)
                    elif mi >= 0:
                        if em_res is not None:
                            emt = em_res[:, mi, :]
                        else:
                            emtt = emt_pool.tile([P, QC], bf16, tag="emt", name="emt")
                            nc.sync.dma_start(out=emtt[:], in_=emask[mi])
                            emt = emtt[:]
                        nc.vector.tensor_tensor(
                            pr[:, zq:], pr[:, zq:], emt[:, zq:], MULT
                        )
                    return pr

                if any(k == "div" for k, _ in epi):
                    rest = deque()
                    while epi:
                        k, fn = epi.popleft()
                        if k == "div":
                            fn()
                        else:
                            rest.append((k, fn))
                    epi.extend(rest)
                # no memsets: the chronologically first matmul into each
                # accumulator bank this qc carries start=True, which marks the
                # whole bank pending-zero -- every region's first write then
                # overwrites stale data (hw has_written semantics; the
                # interpreter models the same bank-granular pending-zero)
                fresh_banks = {"pv01", "pv23", "ms"}

                ns = len(hsteps)
                pscs: dict = {}
                prs: dict = {}
                for s in range(ns):
                    if s == 0:
                        pscs[0] = sc_emit(hsteps[0])
                        if ns > 1:
                            pscs[1] = sc_emit(hsteps[1])
                    if s + 2 < ns:
                        pscs[s + 2] = sc_emit(hsteps[s + 2])
                    prs[s] = exp_emit(hsteps[s], pscs.pop(s))
                    lag = KNOBS["pv_lag"]
                    if s >= lag:
                        h = hsteps[s - lag]
                        drain(("v", h[3]))
                        pv_dn(h[0], h[1], h[2], h[3], h[5], prs.pop(s - lag))
                    if (is_last and KNOBS["aggr_last"]) or s % 2 == 0 or len(epi) + len(fillers) > 8:
                        pop_work()
                for s in range(max(0, ns - KNOBS["pv_lag"]), ns):
                    if s < 0 or s not in prs:
                        continue
                    h = hsteps[s]
                    drain(("v", h[3]))
                    pv_dn(h[0], h[1], h[2], h[3], h[5], prs.pop(s))
                    pop_work()

            def needs(qc):
                res = [("qk", qc // 2, qc % 2)]
                for kt, _, _, _ in plans[qc]:
                    r = ("qk", kt // 8, (kt % 8) // 4)
                    if r not in res:
                        res.append(r)
                return res

            qorder = list(KNOBS["qc_order"])
            for qi, qc in enumerate(qorder):
                for r in needs(qc):
                    drain(r)
                for kt, _, _, _ in plans[qc]:
                    queue(("v", kt))
                if qi + 1 < len(qorder):
                    nqc2 = qorder[qi + 1]
                    for r in needs(nqc2):
                        queue(r)
                    for kt, _, _, _ in plans[nqc2]:
                        queue(("v", kt))
                attention_qc(qc, qi == len(qorder) - 1)
            # final drain: divides/transposes first so the wo chains overlap;
            # wo psum rotates through the now-idle sc banks
            drain_mode[0] = True
            _order = {"div": 0, "tr": 1}
            _rest = sorted(epi, key=lambda kv: _order.get(kv[0], 2))
            epi.clear()
            epi.extend(_rest)
            while epi or fillers:
                pop_work()

    if not nc.is_finalized():
        nc.finalize()
    return nc


def _round_f32r(a):
    """Round fp32 array to the PE's f32r format (mantissa truncated to 11
    bits, round-to-nearest-even at bit 12) so f32r-declared DMA inputs match
    what an on-device cast would produce."""
    u = np.ascontiguousarray(a, dtype=np.float32).view(np.uint32)
    u2 = (u + np.uint32(0x7FF) + ((u >> np.uint32(12)) & np.uint32(1))) & np.uint32(0xFFFFF000)
    return u2.view(np.float32)


def _plan_from_mask(mask):
    """Classify [128, 512] tiles of exp(mask)^T; returns (plans, packed_tiles).

    Entries are (kt, mi, zq, tri): zq (multiple of 128) leading fully-masked
    q-columns; tri=True means the tile is [zeros | tril(128) | ones] so only
    the shared tril block needs multiplying; mi >= 0 indexes a packed general
    bf16 exp(mask) tile.
    """
    em = np.exp(mask.astype(np.float32))  # [q, k]
    emT = np.ascontiguousarray(em.T)  # [k, q]
    # partial diagonal block in [k, q] layout: valid iff q_local >= k_local
    tril_blk = np.triu(np.ones((P, P), dtype=np.float32))
    plans = []
    tiles = []
    tile_keys = {}
    for qc in range(NQC):
        ent = []
        covered = [False] * 4
        for kt in range(NKT):
            t = emT[P * kt : P * (kt + 1), QC * qc : QC * (qc + 1)]
            if not t.any():
                continue  # fully masked out: skip tile entirely
            if (t == 1.0).all():
                ent.append((kt, -1, 0, False))
                continue
            nz = np.flatnonzero(t.any(axis=0))
            zq = (int(nz[0]) // P) * P
            # tril-structured tile: [zeros(zq) | tril | ones]
            tri = (
                zq + P <= QC
                and (t[:, :zq] == 0.0).all()
                and (t[:, zq : zq + P] == tril_blk).all()
                and (t[:, zq + P :] == 1.0).all()
            )
            if tri:
                ent.append((kt, -1, zq, True))
                continue
            key = t.tobytes()
            mi = tile_keys.get(key)
            if mi is None:
                mi = len(tiles)
                tile_keys[key] = mi
                tiles.append(t.astype(ml_dtypes.bfloat16))
            ent.append((kt, mi, zq, False))
        for kt, mi, zq, tri in ent:
            for qt in range(zq // P, 4):
                covered[qt] = True
        if ent and not all(covered):
            # some qt block would never be written: disable skipping (the
            # emask multiply zeroes masked probs so pv/dn stay correct)
            ent2 = []
            for kt, mi, zq, tri in ent:
                if zq == 0:
                    ent2.append((kt, mi, zq, tri))
                    continue
                t = emT[P * kt : P * (kt + 1), QC * qc : QC * (qc + 1)]
                key = t.tobytes()
                mi = tile_keys.get(key)
                if mi is None:
                    mi = len(tiles)
                    tile_keys[key] = mi
                    tiles.append(t.astype(ml_dtypes.bfloat16))
                ent2.append((kt, mi, 0, False))
            ent = ent2
        plans.append(tuple(ent))
    if tiles:
        packed = np.ascontiguousarray(np.stack(tiles))
    else:
        packed = np.zeros((1, P, QC), dtype=ml_dtypes.bfloat16)
    return tuple(plans), packed


def kernel(x, mask, section_log_len, wq, wk, wv, wo, seq_scale):
    from concourse.bass_utils import run_bass_kernel_spmd

    x = np.asarray(x, dtype=np.float32)
    assert x.shape == (B, S, D), x.shape
    mask2 = np.asarray(mask, dtype=np.float32).reshape(S, S)
    sll = np.asarray(section_log_len, dtype=np.float32).reshape(S)
    ss = np.asarray(seq_scale, dtype=np.float32).reshape(H)
    wq = np.asarray(wq, dtype=np.float32)
    wk = np.asarray(wk, dtype=np.float32)
    wv = np.asarray(wv, dtype=np.float32)
    wo = np.asarray(wo, dtype=np.float32)

    plans, tiles = _plan_from_mask(mask2)
    key = (plans, tiles.shape[0])
    nc = _GRAPH_CACHE.get(key)
    if nc is None:
        nc = _build_graph(plans, tiles.shape[0])
        _GRAPH_CACHE[key] = nc

    bf = ml_dtypes.bfloat16
    sllB = np.ascontiguousarray(
        np.broadcast_to(sll[None, :], (P, S)), dtype=np.float32
    )
    xT = [np.ascontiguousarray(x[b].T).astype(bf) for b in range(B)]
    trilB = np.triu(np.ones((P, P), dtype=np.float32)).astype(bf)
    identB = np.eye(P, dtype=np.float32).astype(bf)

    in_maps = []
    for c in range(NCORES):
        b, g2 = divmod(c, 4)
        rows = slice(256 * g2, 256 * (g2 + 1))
        ssr = np.repeat(ss[8 * g2 : 8 * g2 + 8], HD) / math.sqrt(HD)
        in_maps.append(
            {
                "xT": xT[b],
                "aq": np.ascontiguousarray((wq[rows, :] * ssr[:, None]).T).astype(bf),
                "ak": np.ascontiguousarray(wk[rows, :].T).astype(bf),
                "av": np.ascontiguousarray(wv[rows, :].T).astype(bf),
                "wor": np.ascontiguousarray(wo[:, rows].T).astype(bf),
                "sllb": sllB,
                "tril": trilB,
                "ident": identB,
                "emask": tiles,
            }
        )

    res = run_bass_kernel_spmd(nc, in_maps, core_ids=list(range(NCORES))).results
    out = np.zeros((B, S, D), dtype=np.float32)
    for c in range(NCORES):
        out[c // 4] += np.asarray(res[c]["out"], dtype=np.float32)
    return out

